# revision 51
# baseline (speedup 1.0000x reference)
"""Trainium2 Bass kernel for nn_AutoDecoder (moe_routing).

Reference computation (per full input):
  x: [S=3072, B=32, C=512]; rows s%3==1 are "brick" tokens, s%3==2 are
  "combined" tokens (s%3==0 PAD rows are dead). For each (timestep, batch)
  pair:
    brick:  logits[0:80]    = x_brick @ [Ws|Wc]            (+ biases)
    comb:   h = relu(relu(x_comb @ W1 + b1) @ W2 + b2)
            logits[80:1000] = h @ Wh + bh
  out: [TS=1024, B=32, A=1000]

Strategy: data-parallel over batch (4 batch entries per core, 8 cores),
weights replicated. The host pre-packs x into feature-major fp16 tiles
(one [C, ntok] panel per readout name per core), so the device does no
transposes at all: per 512-token block it streams the 2-layer MLP
(fp16 weights, fp32 PSUM accumulation, relu+bias on ACT) and then the
head matmuls with the activations as stationary operands, producing
token-major logits in PSUM. The DVE adds the output bias while
downcasting to fp16, and fully contiguous DMAs write the fp16 logits
back (host upcasts to fp32).

Scheduling, driven by trace analysis: the kernel is PE-bound mid-run
(matmul issue at the measured peak rate: 35/183/206/216ns spacings),
so all slack lives at the edges. Startup is DMA-supply-bound (3 DMA
rings at ~115-130 GB/s each, ~2.4us spin-up, ~4 outstanding DMAs per
HW ring — a 5th issue stalls the issuing engine's FIFO, so the scalar
engine carries exactly the block-0 comb x issues and nothing else
before its ACTIVATEs). The first blocks' heads — the only consumers
of wh — are deferred 2-3 blocks and the PE fills in with MLP-only
work; a dummy-matmul warmup sized to bridge the ring spin-up keeps
the PE continuously busy so the HAM clock reaches 2.4GHz by ~11.5us
with no mid-run re-throttle; the first two blocks run k-outer so
matmuls stream with chunk arrivals, and the trailing 2-wide blocks
run k-outer to avoid ACT-gated matmul pacing. x is packed block-major
on the host so blocks>=2 load each name as ONE contiguous 4KB-line
DMA (the SW-DGE queue is descriptor-bound; strided 1KB lines defeat
its coalescer). Output tiles are written in two column halves on the
two hardware DMA queues; the very last tile computes its upper half
first into a free ps_h psum bank and quarter-splits its output DMAs
across both rings to shorten the drain tail.
"""
import sys

if "/opt/trn_rl_repo" not in sys.path:
    sys.path.append("/opt/trn_rl_repo")

import numpy as np

import concourse.bass as bass
from concourse import bacc
import concourse.mybir as mybir
import concourse.tile as tile
from concourse.bass import ts
from concourse.bass_utils import run_bass_kernel_spmd

F32 = mybir.dt.float32
F16 = mybir.dt.float16
RELU = mybir.ActivationFunctionType.Relu

# problem dims (hardcoded; kernel.py must be self-contained)
S, B, C = 3072, 32, 512
TS_ = S // 3                    # 1024 timesteps
NUM_SHAPES, NUM_COLORS, N_COMBINED = 64, 16, 920
NBRICK = NUM_SHAPES + NUM_COLORS  # 80
A = NBRICK + N_COMBINED           # 1000
NCORES = 8
BL = B // NCORES                  # 4 batch entries per core
NTOKC = TS_ * BL                  # 4096 tokens per name per core
TT = 128                          # tokens per tok-tile
TPB = TT // BL                    # 32 timesteps per tok-tile
KC = C // 128                     # 4 contraction chunks

# block schedule (tok-tiles per block): big first blocks bridge the startup
# DMA supply phase; small last blocks shorten the drain tail. Shared between
# the kernel build and the host-side x pack (the DRAM x layout is
# block-major so each block's per-name load is one contiguous DMA).
SCHED = [4] * 7 + [2, 2]
OFFS = [sum(SCHED[:i]) for i in range(len(SCHED))]
assert sum(SCHED) == TS_ // TPB

_BUILD_CACHE = {}


def _build():
    if "nc" in _BUILD_CACHE:
        return _BUILD_CACHE["nc"]
    nc = bacc.Bacc("TRN2", target_bir_lowering=False, debug=False)

    # feature-major fp16 x, block-major pack: 128 partition rows (feature
    # p within chunk); cols = [name ni][block bi][chunk k][token] so one
    # (name, block) load is a single contiguous DMA with 4KB lines — the
    # SW-DGE queue is descriptor-bound and the old 1KB strided lines
    # defeated its packet coalescer.
    xt_d = nc.declare_dram_parameter("xt", [128, 2 * KC * NTOKC], F16, isOutput=False)
    w1_d = nc.declare_dram_parameter("w1", [C, C], F16, isOutput=False)
    w2_d = nc.declare_dram_parameter("w2", [C, C], F16, isOutput=False)
    wh_d = nc.declare_dram_parameter("wh", [C, N_COMBINED], F16, isOutput=False)
    wsc_d = nc.declare_dram_parameter("wsc", [C, NBRICK], F16, isOutput=False)
    b12_d = nc.declare_dram_parameter("b12", [128, 2 * KC], F32, isOutput=False)
    bA_d = nc.declare_dram_parameter("biasA", [128, A], F32, isOutput=False)
    out_d = nc.declare_dram_parameter("out", [TS_ * BL, A], F16, isOutput=True)

    with tile.TileContext(nc) as tc:
        with (
            tc.tile_pool(name="const", bufs=1) as const,
            tc.tile_pool(name="sb", bufs=4) as sb_p,
            tc.tile_pool(name="psh", bufs=4, space=bass.MemorySpace.PSUM) as ps_h,
            tc.tile_pool(name="psc", bufs=2, space=bass.MemorySpace.PSUM) as ps_c,
        ):
            # one SBUF data pool (buffering is per-tag, so merging pools
            # keeps the same ring depths while shortening the TileContext
            # exit drain chain)
            xt_p = h_p = o_p = sb_p
            # HAM warmup: dummy matmuls at t=0 (on a memset scratch, no DMA
            # dependency) sized to keep the PE continuously busy until the
            # first real weight/x chunks land (~2.3us after the queues spin
            # up), so the HAM clock ramp completes as early as possible and
            # the PE never goes idle before the first real matmul.
            warm_src = const.tile([128, 512], F16, tag="warm")
            nc.vector.memset(warm_src[:], 0.0)
            warm = ps_h.tile([128, 512], F32, tag="hps")
            for _ in range(4):
                nc.tensor.matmul(warm[:, 0:512], warm_src[:, 0:128], warm_src[:])
            for _ in range(10):
                nc.tensor.matmul(warm[:, 0:128], warm_src[:, 0:128], warm_src[:, 0:128])
            # pre-fire the one-time ACT activation-table load so the first
            # real relu doesn't pay ~1.3us for it
            warm_act = const.tile([128, 1], F32, tag="warmact")
            nc.scalar.activation(warm_act[0:1, 0:1], warm_src[0:1, 0:1], RELU)

            # Startup DMA placement (per-HW-queue ~115-130 GB/s for 1KB-line
            # tiles; ~2.4us ring spin-up from first issue to first packet;
            # SW-DGE gpsimd queue coalesces into 4KB packets; HW queues hold
            # only ~4 outstanding DMAs, so a 5th dma_start stalls at the
            # engine's FIFO head until a transfer completes). The scalar
            # engine issues ONLY the 4 block-0 comb x loads and then nothing
            # but ACTIVATEs + out-hi DMAs — any further DMA issue on scalar
            # queues ahead of the critical RELUs in its strict FIFO and
            # stalls layer 2 (a 2.4us PE gap in the baseline trace).
            #   sync:   w1 k0..k3, w2 k0..k3, wh k0..k3, then out-lo
            #   scalar: block-0 comb x (exactly 4 issues — a 5th DMA issue
            #           would stall at the FIFO head on ring backpressure
            #           and block every ACT behind it), then out-hi
            #   gpsimd: b1/b2 (tiny), wsc, block-1 x, block-0 brick x,
            #           biasA, then per-block x prefetch only
            w1_sb = []
            w2_sb = []
            for name, dram, out_list in (("w1", w1_d, w1_sb), ("w2", w2_d, w2_sb)):
                for k in range(KC):
                    t = const.tile([128, C], F16, tag=f"{name}_{k}")
                    out_list.append(t)
            for k in range(KC):
                nc.sync.dma_start(w1_sb[k][:], w1_d[ts(k, 128), :])
            b12_sb = const.tile([128, 2 * KC], F32, tag="b12")
            nc.gpsimd.dma_start(b12_sb[:], b12_d[:, :])
            wh_sb = []
            wsc_sb = []
            for k in range(KC):
                t = const.tile([128, NBRICK], F16, tag=f"wsc_{k}")
                nc.gpsimd.dma_start(t[:], wsc_d[ts(k, 128), :])
                wsc_sb.append(t)
            # w2 k0/k1 ride the SW queue ahead of block-1's x (land ~11-12us)
            # while k2/k3 follow w1 on sync — layer 2 of block 0 is then
            # supplied by ~14.6us instead of ~16.5
            for k in range(2):
                nc.gpsimd.dma_start(w2_sb[k][:], w2_d[ts(k, 128), :])

            def w1s(k, m):
                return w1_sb[k][:, ts(m, 128)]

            def w2s(k, m):
                return w2_sb[k][:, ts(m, 128)]

            def load_late_consts():
                # wh on sync behind w1/w2 (lands ~25.5us, needed at the
                # first finals flush ~27us); biasA DMAs are emitted on the
                # gpsimd SW queue after the early x loads
                for k in range(KC):
                    t = const.tile([128, N_COMBINED], F16, tag=f"wh_{k}")
                    nc.sync.dma_start(t[:], wh_d[ts(k, 128), :])
                    wh_sb.append(t)
                bA_sb = const.tile([128, A], F32, tag="biasA")
                return bA_sb

            # heads for block i, emitted during block i+1: token-major
            # logits straight into PSUM ([0:80] brick, [80:1000] comb),
            # DVE bias-add + fp16 downcast, contiguous DMA out.
            def finals(pb, hi_eng=None):
                # hi_eng: queue for the upper-half out DMA. During the
                # trailing 1-wide blocks the scalar FIFO is the ACT critical
                # chain, so those flushes route hi on sync; the very last
                # tile keeps scalar so the two final transfers parallelize.
                hi_eng = hi_eng or nc.scalar
                last = pb.get("last", False)
                for t in range(pb["nt"]):
                    t_last = last and t == pb["nt"] - 1
                    if t_last:
                        # final tile: its psum halves come from the ps_h
                        # pool (the MLP psums drained already) so the MMs
                        # don't wait for t0's pco to be released by the DVE;
                        # upper comb half first, so its psum stops early and
                        # its bias-add + DMA overlap the lower half's MMs.
                        pco_hi = ps_h.tile([128, 512], F32, tag="hps")
                        pco_lo = ps_h.tile([128, 512], F32, tag="hps")
                        ot = o_p.tile([128, A], F16, tag="osb")
                        r0 = (pb["t0"] + t * TPB) * BL
                        for k in range(KC):
                            nc.tensor.matmul(
                                pco_hi[:, 0 : A - 512],
                                pb["h2"][k][:, ts(t, 128)],
                                wh_sb[k][:, 512 - NBRICK : N_COMBINED],
                                start=(k == 0),
                                stop=(k == KC - 1),
                            )
                        nc.vector.tensor_add(
                            ot[:, 512:A], pco_hi[:, 0 : A - 512], bA_sb[:, 512:A]
                        )
                        # quarter-split out DMAs across both rings so the
                        # post-last-matmul transfer is ~64KB per ring, not
                        # one 128KB straggler on a single ring
                        nc.scalar.dma_start(out_d[r0 : r0 + TT, 512:768], ot[:, 512:768])
                        nc.sync.dma_start(out_d[r0 : r0 + TT, 768:A], ot[:, 768:A])
                        for k in range(KC):
                            nc.tensor.matmul(
                                pco_lo[:, 0:NBRICK],
                                pb["xtb"][k][:, ts(t, 128)],
                                wsc_sb[k][:],
                                start=(k == 0),
                                stop=(k == KC - 1),
                            )
                        for k in range(KC):
                            nc.tensor.matmul(
                                pco_lo[:, NBRICK:512],
                                pb["h2"][k][:, ts(t, 128)],
                                wh_sb[k][:, 0 : 512 - NBRICK],
                                start=(k == 0),
                                stop=(k == KC - 1),
                            )
                        # lo half in quarter adds so each ring's final DMA
                        # issues as soon as its quarter is biased
                        nc.vector.tensor_add(
                            ot[:, 0:256], pco_lo[:, 0:256], bA_sb[:, 0:256]
                        )
                        nc.scalar.dma_start(out_d[r0 : r0 + TT, 0:256], ot[:, 0:256])
                        nc.vector.tensor_add(
                            ot[:, 256:512], pco_lo[:, 256:512], bA_sb[:, 256:512]
                        )
                        nc.sync.dma_start(out_d[r0 : r0 + TT, 256:512], ot[:, 256:512])
                        continue
                    pco = ps_c.tile([128, 1024], F32, tag="combo")
                    for k in range(KC):
                        lhs = pb["h2"][k][:, ts(t, 128)]
                        nc.tensor.matmul(
                            pco[:, NBRICK:512],
                            lhs,
                            wh_sb[k][:, 0 : 512 - NBRICK],
                            start=(k == 0),
                            stop=(k == KC - 1),
                        )
                        nc.tensor.matmul(
                            pco[:, 512:A],
                            lhs,
                            wh_sb[k][:, 512 - NBRICK : N_COMBINED],
                            start=(k == 0),
                            stop=(k == KC - 1),
                        )
                    for k in range(KC):
                        nc.tensor.matmul(
                            pco[:, 0:NBRICK],
                            pb["xtb"][k][:, ts(t, 128)],
                            wsc_sb[k][:],
                            start=(k == 0),
                            stop=(k == KC - 1),
                        )
                    # bias-add + fp16 downcast in two column halves so the
                    # first half's DMA fires while the DVE does the second
                    # half; halves go to different queues
                    ot = o_p.tile([128, A], F16, tag="osb")
                    r0 = (pb["t0"] + t * TPB) * BL
                    nc.vector.tensor_add(ot[:, 0:512], pco[:, 0:512], bA_sb[:, 0:512])
                    nc.sync.dma_start(out_d[r0 : r0 + TT, 0:512], ot[:, 0:512])
                    nc.vector.tensor_add(ot[:, 512:A], pco[:, 512:A], bA_sb[:, 512:A])
                    hi_eng.dma_start(out_d[r0 : r0 + TT, 512:A], ot[:, 512:A])

            # ---- main loop over blocks of tok-tiles (128 tokens each) ----
            sched = SCHED
            offs = OFFS
            # heads deferral: the first ~25us are DMA-limited (weights still
            # streaming in at ~60-80 GB/s per queue), so the first blocks'
            # heads — which need wh — are pushed back 1-2 extra blocks; the
            # PE fills that time with MLP work that only needs w1/w2/x. The
            # backlog is worked off two-per-block once wh has landed.
            nblk = len(sched)
            flush_n = {2: 1, 3: 2}
            flush_n.update({i: 1 for i in range(4, nblk)})

            # feature-major fp16 x loads, issued one block AHEAD so the
            # queue has a full block period to deliver. Block 0's comb x
            # goes on the scalar HW queue (parallel with w1 on sync) and
            # its brick x on gpsimd; later blocks use the gpsimd
            # software-DGE queue entirely. Blocks 0/1 split per k-chunk
            # (compute streams with chunk arrivals; block 0's first chunk
            # is halved again for the earliest possible first matmul);
            # later blocks load each name as ONE contiguous 4KB-line DMA.
            def issue_name_loads(bi, ni, eng):
                nt = sched[bi]
                W_ = nt * TT
                c0 = ni * (KC * NTOKC) + offs[bi] * TT * KC
                tl = xt_p.tile([128, KC * W_], F16, tag=f"xt{ni}")
                if bi == 0 and ni == 1:
                    eng.dma_start(tl[:, 0:W_], xt_d[:, c0 : c0 + W_])
                    eng.dma_start(tl[:, W_ : 2 * W_], xt_d[:, c0 + W_ : c0 + 2 * W_])
                    eng.dma_start(
                        tl[:, 2 * W_ : 4 * W_], xt_d[:, c0 + 2 * W_ : c0 + 4 * W_]
                    )
                elif bi == 1:
                    for k in range(KC):
                        eng.dma_start(
                            tl[:, k * W_ : (k + 1) * W_],
                            xt_d[:, c0 + k * W_ : c0 + (k + 1) * W_],
                        )
                else:
                    eng.dma_start(tl[:], xt_d[:, c0 : c0 + KC * W_])
                return [tl[:, k * W_ : (k + 1) * W_] for k in range(KC)]

            def issue_loads(bi):
                return (
                    issue_name_loads(bi, 1, nc.gpsimd),
                    issue_name_loads(bi, 0, nc.gpsimd),
                )

            xtc0 = issue_name_loads(0, 1, nc.scalar)
            for k in range(2, KC):
                nc.sync.dma_start(w2_sb[k][:], w2_d[ts(k, 128), :])
            bA_sb = load_late_consts()
            issued = {1: issue_loads(1)}
            # block 0's brick x goes behind block 1's x on the SW queue —
            # it is not read until the deferred heads flush
            issued[0] = (xtc0, issue_name_loads(0, 0, nc.gpsimd))
            # biasA rides the deep SW-DGE ring behind the early x loads
            # (lands ~22us, first bias-add needs it ~27us); the HW rings
            # would either stall ACTs (scalar) or push wh too late (sync)
            nc.gpsimd.dma_start(bA_sb[:, 0:512], bA_d[:, 0:512])
            nc.gpsimd.dma_start(bA_sb[:, 512:A], bA_d[:, 512:A])
            pendings = []
            for bi, nt in enumerate(sched):
                ti0 = offs[bi]
                t0 = ti0 * TPB
                W_ = nt * TT  # tokens per name in this block
                xtc, xtb = issued.pop(bi)
                if bi + 1 < len(sched) and bi + 1 not in issued:
                    issued[bi + 1] = issue_loads(bi + 1)

                # deferred heads of earlier blocks (see note above)
                for _ in range(flush_n.get(bi, 0)):
                    finals(
                        pendings.pop(0),
                        hi_eng=nc.sync if bi >= nblk - 2 else nc.scalar,
                    )

                # comb MLP: hT[m] = relu(W[:,m-chunk].T @ inT + b). The first
                # two blocks run k-outer so matmuls stream with the arriving
                # weight/x DMA chunks; the trailing 1-wide blocks run k-outer
                # too (m-outer at W=128 is ACT-gated: ~190ns/MM instead of
                # 56); mid-run 4-wide blocks run m-outer so the ACT engine
                # drains each m-psum while the next accumulates.
                def layer(ws, inp, btag, hname):
                    hs_list = []
                    if bi < 2 or bi >= nblk - 2:
                        phs = []
                        for _m in range(KC):
                            ph = ps_h.tile([128, W_], F32, tag="hps")
                            phs.append(ph)
                        # NOTE: do not split a k-stage into column sub-ranges
                        # of one psum bank — a bank supports only ONE open
                        # accumulation group at a time, and a second start=
                        # True group in the same bank corrupts the first.
                        for k in range(KC):
                            for m in range(KC):
                                nc.tensor.matmul(
                                    phs[m][:],
                                    ws(k, m),
                                    inp[k][:],
                                    start=(k == 0),
                                    stop=(k == KC - 1),
                                )
                        for m in range(KC):
                            hs = h_p.tile([128, W_], F16, tag=f"{hname}_{m}")
                            nc.scalar.activation(
                                hs[:],
                                phs[m][:],
                                RELU,
                                bias=b12_sb[:, btag + m : btag + m + 1],
                                scale=1.0,
                            )
                            hs_list.append(hs)
                    else:
                        for m in range(KC):
                            ph = ps_h.tile([128, W_], F32, tag="hps")
                            for k in range(KC):
                                nc.tensor.matmul(
                                    ph[:],
                                    ws(k, m),
                                    inp[k][:],
                                    start=(k == 0),
                                    stop=(k == KC - 1),
                                )
                            hs = h_p.tile([128, W_], F16, tag=f"{hname}_{m}")
                            nc.scalar.activation(
                                hs[:],
                                ph[:],
                                RELU,
                                bias=b12_sb[:, btag + m : btag + m + 1],
                                scale=1.0,
                            )
                            hs_list.append(hs)
                    return hs_list

                h1 = layer(w1s, xtc, 0, "h1")
                h2 = layer(w2s, h1, KC, "h2")

                pendings.append({"h2": h2, "xtb": xtb, "t0": t0, "nt": nt})
            assert len(pendings) == 1
            pendings[0]["last"] = True
            # keep-warm: the out rings have been idle for ~5us by now; a
            # tiny read on each re-spins them so the final out transfers
            # don't pay the ring restart latency
            kw = const.tile([1, C], F16, tag="kw")
            nc.sync.dma_start(kw[0:1, 0:256], w1_d[0:1, 0:256])
            nc.scalar.dma_start(kw[0:1, 256:512], w1_d[0:1, 256:512])
            finals(pendings[0])

    nc.compile()
    _BUILD_CACHE["nc"] = nc
    return nc


def _prepare_inputs(inputs):
    """Host-side prep: validate/normalize routing, shard over batch,
    pre-pack x feature-major fp16, replicate weights. Returns in_maps
    for the 8 cores."""
    x = np.ascontiguousarray(np.asarray(inputs["x"], dtype=np.float32))
    readout_x = np.asarray(inputs["readout_x"], dtype=np.int32)
    W1 = np.asarray(inputs["W1"], dtype=np.float32)
    W2 = np.asarray(inputs["W2"], dtype=np.float32)
    Wh = np.asarray(inputs["Wh"], dtype=np.float32)
    Ws = np.asarray(inputs["Ws"], dtype=np.float32)
    Wc = np.asarray(inputs["Wc"], dtype=np.float32)
    b1 = np.asarray(inputs["b1"], dtype=np.float32)
    b2 = np.asarray(inputs["b2"], dtype=np.float32)
    bh = np.asarray(inputs["bh"], dtype=np.float32)
    bs = np.asarray(inputs["bs"], dtype=np.float32)
    bc = np.asarray(inputs["bc"], dtype=np.float32)

    # The kernel hardcodes the cyclic PAD/brick/comb routing. If the actual
    # readout pattern differs, permute x on the host so the device sees the
    # canonical layout (mirrors jnp.nonzero(..., size=ntok) semantics).
    ntok = TS_ * B
    rf = readout_x.reshape(-1)
    canonical = np.array_equal(
        readout_x, np.broadcast_to((np.arange(S, dtype=np.int32) % 3)[:, None], (S, B))
    )
    if not canonical:
        xf = x.reshape(S * B, C)
        xc = np.zeros_like(x).reshape(S * B, C)
        for name_idx in (1, 2):
            idx = np.nonzero(rf == name_idx)[0]
            if idx.shape[0] < ntok:
                idx = np.pad(idx, (0, ntok - idx.shape[0]))
            else:
                idx = idx[:ntok]
            tgt = (3 * (np.arange(ntok) // B) + name_idx) * B + (np.arange(ntok) % B)
            xc[tgt] = xf[idx]
        x = xc.reshape(S, B, C)

    # feature-major fp16 pack: [name, core, C, t, b] with brick first, then
    # block-major cols per core: [p, ni, block, k, tok] so each (name,
    # block) is one contiguous 4KB-line DMA
    xh = x.astype(np.float16).reshape(TS_, 3, NCORES, BL, C)
    xp = xh[:, 1:3].transpose(1, 2, 4, 0, 3)  # [name, core, C, t, b]

    Wsc = np.ascontiguousarray(np.concatenate([Ws, Wc], axis=1).astype(np.float16))
    W1h = np.ascontiguousarray(W1.astype(np.float16))
    W2h = np.ascontiguousarray(W2.astype(np.float16))
    Whh = np.ascontiguousarray(Wh.astype(np.float16))
    b12 = np.ascontiguousarray(
        np.concatenate([b1.reshape(KC, 128).T, b2.reshape(KC, 128).T], axis=1)
    )
    biasA = np.concatenate([bs, bc, bh])
    biasA_b = np.ascontiguousarray(np.broadcast_to(biasA, (128, A)))

    in_maps = []
    for c in range(NCORES):
        # brick ni=0 from readout name 1, comb ni=1 from name 2
        arr = xp[:, c].reshape(2, KC, 128, TS_, BL)  # [ni, k, p, t, b]
        segs = []
        for ni in range(2):
            for bi, nt in enumerate(SCHED):
                t0 = OFFS[bi] * TPB
                seg = arr[ni, :, :, t0 : t0 + nt * TPB, :]  # [k, p, tt, b]
                segs.append(
                    seg.transpose(1, 0, 2, 3).reshape(128, KC * nt * TT)
                )
        xt_core = np.ascontiguousarray(np.concatenate(segs, axis=1))
        in_maps.append(
            {
                "xt": xt_core,
                "w1": W1h,
                "w2": W2h,
                "wh": Whh,
                "wsc": Wsc,
                "b12": b12,
                "biasA": biasA_b,
            }
        )
    return in_maps


def _run(inputs, trace=False, trace_kwargs=None):
    nc = _build()
    in_maps = _prepare_inputs(inputs)
    res = run_bass_kernel_spmd(
        nc,
        in_maps,
        list(range(NCORES)),
        trace=trace,
        **(trace_kwargs or {}),
    )
    out = np.empty((TS_, B, A), dtype=np.float32)
    for c in range(NCORES):
        out[:, c * BL : (c + 1) * BL, :] = (
            res.results[c]["out"].reshape(TS_, BL, A).astype(np.float32)
        )
    return out, res


def kernel(**inputs) -> np.ndarray:
    out, _ = _run(inputs, trace=False)
    return out


if __name__ == "__main__":
    nc = _build()
    print("built OK")



# revision 52
# speedup vs baseline: 1.0099x; 1.0099x over previous
"""Trainium2 Bass kernel for nn_AutoDecoder (moe_routing).

Reference computation (per full input):
  x: [S=3072, B=32, C=512]; rows s%3==1 are "brick" tokens, s%3==2 are
  "combined" tokens (s%3==0 PAD rows are dead). For each (timestep, batch)
  pair:
    brick:  logits[0:80]    = x_brick @ [Ws|Wc]            (+ biases)
    comb:   h = relu(relu(x_comb @ W1 + b1) @ W2 + b2)
            logits[80:1000] = h @ Wh + bh
  out: [TS=1024, B=32, A=1000]

Strategy: data-parallel over batch (4 batch entries per core, 8 cores),
weights replicated. The host pre-packs x into feature-major fp16 tiles
(one [C, ntok] panel per readout name per core), so the device does no
transposes at all: per 512-token block it streams the 2-layer MLP
(fp16 weights, fp32 PSUM accumulation, relu+bias on ACT) and then the
head matmuls with the activations as stationary operands, producing
token-major logits in PSUM. The DVE adds the output bias while
downcasting to fp16, and fully contiguous DMAs write the fp16 logits
back (host upcasts to fp32).

Scheduling, driven by trace analysis: the kernel is PE-bound mid-run
(matmul issue at the measured peak rate: 35/183/206/216ns spacings),
so all slack lives at the edges. Startup is DMA-supply-bound (3 DMA
rings at ~115-130 GB/s each, ~2.4us spin-up, ~4 outstanding DMAs per
HW ring — a 5th issue stalls the issuing engine's FIFO, so the scalar
engine carries exactly the block-0 comb x issues and nothing else
before its ACTIVATEs). The first blocks' heads — the only consumers
of wh — are deferred 2-3 blocks and the PE fills in with MLP-only
work; a dummy-matmul warmup sized to bridge the ring spin-up keeps
the PE continuously busy so the HAM clock reaches 2.4GHz by ~11.5us
with no mid-run re-throttle; the first two blocks run k-outer so
matmuls stream with chunk arrivals, and the trailing 2-wide blocks
run k-outer to avoid ACT-gated matmul pacing. x is packed block-major
on the host so blocks>=2 load each name as ONE contiguous 4KB-line
DMA (the SW-DGE queue is descriptor-bound; strided 1KB lines defeat
its coalescer). Output tiles are written in two column halves on the
two hardware DMA queues; the very last tile computes its upper half
first into a free ps_h psum bank and quarter-splits its output DMAs
across both rings to shorten the drain tail.
"""
import sys

if "/opt/trn_rl_repo" not in sys.path:
    sys.path.append("/opt/trn_rl_repo")

import numpy as np

import concourse.bass as bass
from concourse import bacc
import concourse.mybir as mybir
import concourse.tile as tile
from concourse.bass import ts
from concourse.bass_utils import run_bass_kernel_spmd

F32 = mybir.dt.float32
F16 = mybir.dt.float16
RELU = mybir.ActivationFunctionType.Relu

# problem dims (hardcoded; kernel.py must be self-contained)
S, B, C = 3072, 32, 512
TS_ = S // 3                    # 1024 timesteps
NUM_SHAPES, NUM_COLORS, N_COMBINED = 64, 16, 920
NBRICK = NUM_SHAPES + NUM_COLORS  # 80
A = NBRICK + N_COMBINED           # 1000
NCORES = 8
BL = B // NCORES                  # 4 batch entries per core
NTOKC = TS_ * BL                  # 4096 tokens per name per core
TT = 128                          # tokens per tok-tile
TPB = TT // BL                    # 32 timesteps per tok-tile
KC = C // 128                     # 4 contraction chunks

# block schedule (tok-tiles per block): big first blocks bridge the startup
# DMA supply phase; small last blocks shorten the drain tail. Shared between
# the kernel build and the host-side x pack (the DRAM x layout is
# block-major so each block's per-name load is one contiguous DMA).
SCHED = [4] * 7 + [2, 2]
OFFS = [sum(SCHED[:i]) for i in range(len(SCHED))]
assert sum(SCHED) == TS_ // TPB

_BUILD_CACHE = {}


def _build():
    if "nc" in _BUILD_CACHE:
        return _BUILD_CACHE["nc"]
    nc = bacc.Bacc("TRN2", target_bir_lowering=False, debug=False)

    # feature-major fp16 x, block-major pack: 128 partition rows (feature
    # p within chunk); cols = [name ni][block bi][chunk k][token] so one
    # (name, block) load is a single contiguous DMA with 4KB lines — the
    # SW-DGE queue is descriptor-bound and the old 1KB strided lines
    # defeated its packet coalescer.
    xt_d = nc.declare_dram_parameter("xt", [128, 2 * KC * NTOKC], F16, isOutput=False)
    w1_d = nc.declare_dram_parameter("w1", [C, C], F16, isOutput=False)
    w2_d = nc.declare_dram_parameter("w2", [C, C], F16, isOutput=False)
    wh_d = nc.declare_dram_parameter("wh", [C, N_COMBINED], F16, isOutput=False)
    wsc_d = nc.declare_dram_parameter("wsc", [C, NBRICK], F16, isOutput=False)
    b12_d = nc.declare_dram_parameter("b12", [128, 2 * KC], F32, isOutput=False)
    bA_d = nc.declare_dram_parameter("biasA", [128, A], F32, isOutput=False)
    out_d = nc.declare_dram_parameter("out", [TS_ * BL, A], F16, isOutput=True)

    with tile.TileContext(nc) as tc:
        with (
            tc.tile_pool(name="const", bufs=1) as const,
            tc.tile_pool(name="xt", bufs=4) as xt_p,
            tc.tile_pool(name="h", bufs=4) as h_p,
            tc.tile_pool(name="osb", bufs=4) as o_p,
            tc.tile_pool(name="psh", bufs=4, space=bass.MemorySpace.PSUM) as ps_h,
            tc.tile_pool(name="psc", bufs=2, space=bass.MemorySpace.PSUM) as ps_c,
        ):
            # HAM warmup: dummy matmuls at t=0 (on a memset scratch, no DMA
            # dependency) sized to keep the PE continuously busy until the
            # first real weight/x chunks land (~2.3us after the queues spin
            # up), so the HAM clock ramp completes as early as possible and
            # the PE never goes idle before the first real matmul.
            warm_src = const.tile([128, 512], F16, tag="warm")
            nc.vector.memset(warm_src[:], 0.0)
            warm = ps_h.tile([128, 512], F32, tag="hps")
            for _ in range(4):
                nc.tensor.matmul(warm[:, 0:512], warm_src[:, 0:128], warm_src[:])
            for _ in range(10):
                nc.tensor.matmul(warm[:, 0:128], warm_src[:, 0:128], warm_src[:, 0:128])
            # pre-fire the one-time ACT activation-table load so the first
            # real relu doesn't pay ~1.3us for it
            warm_act = const.tile([128, 1], F32, tag="warmact")
            nc.scalar.activation(warm_act[0:1, 0:1], warm_src[0:1, 0:1], RELU)

            # Startup DMA placement (per-HW-queue ~115-130 GB/s for 1KB-line
            # tiles; ~2.4us ring spin-up from first issue to first packet;
            # SW-DGE gpsimd queue coalesces into 4KB packets; HW queues hold
            # only ~4 outstanding DMAs, so a 5th dma_start stalls at the
            # engine's FIFO head until a transfer completes). The scalar
            # engine issues ONLY the 4 block-0 comb x loads and then nothing
            # but ACTIVATEs + out-hi DMAs — any further DMA issue on scalar
            # queues ahead of the critical RELUs in its strict FIFO and
            # stalls layer 2 (a 2.4us PE gap in the baseline trace).
            #   sync:   w1 k0..k3, w2 k0..k3, wh k0..k3, then out-lo
            #   scalar: block-0 comb x (exactly 4 issues — a 5th DMA issue
            #           would stall at the FIFO head on ring backpressure
            #           and block every ACT behind it), then out-hi
            #   gpsimd: b1/b2 (tiny), wsc, block-1 x, block-0 brick x,
            #           biasA, then per-block x prefetch only
            w1_sb = []
            w2_sb = []
            for name, dram, out_list in (("w1", w1_d, w1_sb), ("w2", w2_d, w2_sb)):
                for k in range(KC):
                    t = const.tile([128, C], F16, tag=f"{name}_{k}")
                    out_list.append(t)
            for k in range(KC):
                nc.sync.dma_start(w1_sb[k][:], w1_d[ts(k, 128), :])
            b12_sb = const.tile([128, 2 * KC], F32, tag="b12")
            nc.gpsimd.dma_start(b12_sb[:], b12_d[:, :])
            wh_sb = []
            wsc_sb = []
            for k in range(KC):
                t = const.tile([128, NBRICK], F16, tag=f"wsc_{k}")
                nc.gpsimd.dma_start(t[:], wsc_d[ts(k, 128), :])
                wsc_sb.append(t)
            # w2 k0/k1 ride the SW queue ahead of block-1's x (land ~11-12us)
            # while k2/k3 follow w1 on sync — layer 2 of block 0 is then
            # supplied by ~14.6us instead of ~16.5
            for k in range(2):
                nc.gpsimd.dma_start(w2_sb[k][:], w2_d[ts(k, 128), :])

            def w1s(k, m):
                return w1_sb[k][:, ts(m, 128)]

            def w2s(k, m):
                return w2_sb[k][:, ts(m, 128)]

            def load_late_consts():
                # wh on sync behind w1/w2 (lands ~25.5us, needed at the
                # first finals flush ~27us); biasA DMAs are emitted on the
                # gpsimd SW queue after the early x loads
                for k in range(KC):
                    t = const.tile([128, N_COMBINED], F16, tag=f"wh_{k}")
                    nc.sync.dma_start(t[:], wh_d[ts(k, 128), :])
                    wh_sb.append(t)
                bA_sb = const.tile([128, A], F32, tag="biasA")
                return bA_sb

            # heads for block i, emitted during block i+1: token-major
            # logits straight into PSUM ([0:80] brick, [80:1000] comb),
            # DVE bias-add + fp16 downcast, contiguous DMA out.
            def finals(pb, hi_eng=None):
                # hi_eng: queue for the upper-half out DMA. During the
                # trailing 1-wide blocks the scalar FIFO is the ACT critical
                # chain, so those flushes route hi on sync; the very last
                # tile keeps scalar so the two final transfers parallelize.
                hi_eng = hi_eng or nc.scalar
                last = pb.get("last", False)
                for t in range(pb["nt"]):
                    t_last = last and t == pb["nt"] - 1
                    if t_last:
                        # final tile: its psum halves come from the ps_h
                        # pool (the MLP psums drained already) so the MMs
                        # don't wait for t0's pco to be released by the DVE;
                        # upper comb half first, so its psum stops early and
                        # its bias-add + DMA overlap the lower half's MMs.
                        pco_hi = ps_h.tile([128, 512], F32, tag="hps")
                        pco_lo = ps_h.tile([128, 512], F32, tag="hps")
                        ot = o_p.tile([128, A], F16, tag="osb")
                        r0 = (pb["t0"] + t * TPB) * BL
                        for k in range(KC):
                            nc.tensor.matmul(
                                pco_hi[:, 0 : A - 512],
                                pb["h2"][k][:, ts(t, 128)],
                                wh_sb[k][:, 512 - NBRICK : N_COMBINED],
                                start=(k == 0),
                                stop=(k == KC - 1),
                            )
                        nc.vector.tensor_add(
                            ot[:, 512:A], pco_hi[:, 0 : A - 512], bA_sb[:, 512:A]
                        )
                        # quarter-split out DMAs across both rings so the
                        # post-last-matmul transfer is ~64KB per ring, not
                        # one 128KB straggler on a single ring
                        nc.scalar.dma_start(out_d[r0 : r0 + TT, 512:768], ot[:, 512:768])
                        nc.sync.dma_start(out_d[r0 : r0 + TT, 768:A], ot[:, 768:A])
                        for k in range(KC):
                            nc.tensor.matmul(
                                pco_lo[:, 0:NBRICK],
                                pb["xtb"][k][:, ts(t, 128)],
                                wsc_sb[k][:],
                                start=(k == 0),
                                stop=(k == KC - 1),
                            )
                        for k in range(KC):
                            nc.tensor.matmul(
                                pco_lo[:, NBRICK:512],
                                pb["h2"][k][:, ts(t, 128)],
                                wh_sb[k][:, 0 : 512 - NBRICK],
                                start=(k == 0),
                                stop=(k == KC - 1),
                            )
                        # lo half in quarter adds so each ring's final DMA
                        # issues as soon as its quarter is biased
                        nc.vector.tensor_add(
                            ot[:, 0:256], pco_lo[:, 0:256], bA_sb[:, 0:256]
                        )
                        nc.scalar.dma_start(out_d[r0 : r0 + TT, 0:256], ot[:, 0:256])
                        nc.vector.tensor_add(
                            ot[:, 256:512], pco_lo[:, 256:512], bA_sb[:, 256:512]
                        )
                        nc.sync.dma_start(out_d[r0 : r0 + TT, 256:512], ot[:, 256:512])
                        continue
                    pco = ps_c.tile([128, 1024], F32, tag="combo")
                    for k in range(KC):
                        lhs = pb["h2"][k][:, ts(t, 128)]
                        nc.tensor.matmul(
                            pco[:, NBRICK:512],
                            lhs,
                            wh_sb[k][:, 0 : 512 - NBRICK],
                            start=(k == 0),
                            stop=(k == KC - 1),
                        )
                        nc.tensor.matmul(
                            pco[:, 512:A],
                            lhs,
                            wh_sb[k][:, 512 - NBRICK : N_COMBINED],
                            start=(k == 0),
                            stop=(k == KC - 1),
                        )
                    for k in range(KC):
                        nc.tensor.matmul(
                            pco[:, 0:NBRICK],
                            pb["xtb"][k][:, ts(t, 128)],
                            wsc_sb[k][:],
                            start=(k == 0),
                            stop=(k == KC - 1),
                        )
                    # bias-add + fp16 downcast in two column halves so the
                    # first half's DMA fires while the DVE does the second
                    # half; halves go to different queues
                    ot = o_p.tile([128, A], F16, tag="osb")
                    r0 = (pb["t0"] + t * TPB) * BL
                    nc.vector.tensor_add(ot[:, 0:512], pco[:, 0:512], bA_sb[:, 0:512])
                    nc.sync.dma_start(out_d[r0 : r0 + TT, 0:512], ot[:, 0:512])
                    nc.vector.tensor_add(ot[:, 512:A], pco[:, 512:A], bA_sb[:, 512:A])
                    hi_eng.dma_start(out_d[r0 : r0 + TT, 512:A], ot[:, 512:A])

            # ---- main loop over blocks of tok-tiles (128 tokens each) ----
            sched = SCHED
            offs = OFFS
            # heads deferral: the first ~25us are DMA-limited (weights still
            # streaming in at ~60-80 GB/s per queue), so the first blocks'
            # heads — which need wh — are pushed back 1-2 extra blocks; the
            # PE fills that time with MLP work that only needs w1/w2/x. The
            # backlog is worked off two-per-block once wh has landed.
            nblk = len(sched)
            flush_n = {2: 1, 3: 2}
            flush_n.update({i: 1 for i in range(4, nblk)})

            # feature-major fp16 x loads, issued one block AHEAD so the
            # queue has a full block period to deliver. Block 0's comb x
            # goes on the scalar HW queue (parallel with w1 on sync) and
            # its brick x on gpsimd; later blocks use the gpsimd
            # software-DGE queue entirely. Blocks 0/1 split per k-chunk
            # (compute streams with chunk arrivals; block 0's first chunk
            # is halved again for the earliest possible first matmul);
            # later blocks load each name as ONE contiguous 4KB-line DMA.
            def issue_name_loads(bi, ni, eng):
                nt = sched[bi]
                W_ = nt * TT
                c0 = ni * (KC * NTOKC) + offs[bi] * TT * KC
                tl = xt_p.tile([128, KC * W_], F16, tag=f"xt{ni}")
                if bi == 0 and ni == 1:
                    eng.dma_start(tl[:, 0:W_], xt_d[:, c0 : c0 + W_])
                    eng.dma_start(tl[:, W_ : 2 * W_], xt_d[:, c0 + W_ : c0 + 2 * W_])
                    eng.dma_start(
                        tl[:, 2 * W_ : 4 * W_], xt_d[:, c0 + 2 * W_ : c0 + 4 * W_]
                    )
                elif bi == 1:
                    for k in range(KC):
                        eng.dma_start(
                            tl[:, k * W_ : (k + 1) * W_],
                            xt_d[:, c0 + k * W_ : c0 + (k + 1) * W_],
                        )
                else:
                    eng.dma_start(tl[:], xt_d[:, c0 : c0 + KC * W_])
                return [tl[:, k * W_ : (k + 1) * W_] for k in range(KC)]

            def issue_loads(bi):
                return (
                    issue_name_loads(bi, 1, nc.gpsimd),
                    issue_name_loads(bi, 0, nc.gpsimd),
                )

            xtc0 = issue_name_loads(0, 1, nc.scalar)
            for k in range(2, KC):
                nc.sync.dma_start(w2_sb[k][:], w2_d[ts(k, 128), :])
            bA_sb = load_late_consts()
            issued = {1: issue_loads(1)}
            # block 0's brick x goes behind block 1's x on the SW queue —
            # it is not read until the deferred heads flush
            issued[0] = (xtc0, issue_name_loads(0, 0, nc.gpsimd))
            # biasA rides the deep SW-DGE ring behind the early x loads
            # (lands ~22us, first bias-add needs it ~27us); the HW rings
            # would either stall ACTs (scalar) or push wh too late (sync)
            nc.gpsimd.dma_start(bA_sb[:, 0:512], bA_d[:, 0:512])
            nc.gpsimd.dma_start(bA_sb[:, 512:A], bA_d[:, 512:A])
            pendings = []
            for bi, nt in enumerate(sched):
                ti0 = offs[bi]
                t0 = ti0 * TPB
                W_ = nt * TT  # tokens per name in this block
                xtc, xtb = issued.pop(bi)
                if bi + 1 < len(sched) and bi + 1 not in issued:
                    issued[bi + 1] = issue_loads(bi + 1)

                # deferred heads of earlier blocks (see note above)
                for _ in range(flush_n.get(bi, 0)):
                    finals(
                        pendings.pop(0),
                        hi_eng=nc.sync if bi >= nblk - 2 else nc.scalar,
                    )

                # comb MLP: hT[m] = relu(W[:,m-chunk].T @ inT + b). The first
                # two blocks run k-outer so matmuls stream with the arriving
                # weight/x DMA chunks; the trailing 1-wide blocks run k-outer
                # too (m-outer at W=128 is ACT-gated: ~190ns/MM instead of
                # 56); mid-run 4-wide blocks run m-outer so the ACT engine
                # drains each m-psum while the next accumulates.
                def layer(ws, inp, btag, hname):
                    hs_list = []
                    if bi < 2 or bi >= nblk - 2:
                        phs = []
                        for _m in range(KC):
                            ph = ps_h.tile([128, W_], F32, tag="hps")
                            phs.append(ph)
                        # NOTE: do not split a k-stage into column sub-ranges
                        # of one psum bank — a bank supports only ONE open
                        # accumulation group at a time, and a second start=
                        # True group in the same bank corrupts the first.
                        for k in range(KC):
                            for m in range(KC):
                                nc.tensor.matmul(
                                    phs[m][:],
                                    ws(k, m),
                                    inp[k][:],
                                    start=(k == 0),
                                    stop=(k == KC - 1),
                                )
                        for m in range(KC):
                            hs = h_p.tile([128, W_], F16, tag=f"{hname}_{m}")
                            nc.scalar.activation(
                                hs[:],
                                phs[m][:],
                                RELU,
                                bias=b12_sb[:, btag + m : btag + m + 1],
                                scale=1.0,
                            )
                            hs_list.append(hs)
                    else:
                        for m in range(KC):
                            ph = ps_h.tile([128, W_], F32, tag="hps")
                            for k in range(KC):
                                nc.tensor.matmul(
                                    ph[:],
                                    ws(k, m),
                                    inp[k][:],
                                    start=(k == 0),
                                    stop=(k == KC - 1),
                                )
                            hs = h_p.tile([128, W_], F16, tag=f"{hname}_{m}")
                            nc.scalar.activation(
                                hs[:],
                                ph[:],
                                RELU,
                                bias=b12_sb[:, btag + m : btag + m + 1],
                                scale=1.0,
                            )
                            hs_list.append(hs)
                    return hs_list

                h1 = layer(w1s, xtc, 0, "h1")
                h2 = layer(w2s, h1, KC, "h2")

                pendings.append({"h2": h2, "xtb": xtb, "t0": t0, "nt": nt})
            assert len(pendings) == 1
            pendings[0]["last"] = True
            # keep-warm: the out rings have been idle for ~5us by now; a
            # tiny read on each re-spins them so the final out transfers
            # don't pay the ring restart latency
            kw = const.tile([1, C], F16, tag="kw")
            nc.sync.dma_start(kw[0:1, 0:256], w1_d[0:1, 0:256])
            nc.scalar.dma_start(kw[0:1, 256:512], w1_d[0:1, 256:512])
            finals(pendings[0])

    nc.compile()
    _BUILD_CACHE["nc"] = nc
    return nc


def _prepare_inputs(inputs):
    """Host-side prep: validate/normalize routing, shard over batch,
    pre-pack x feature-major fp16, replicate weights. Returns in_maps
    for the 8 cores."""
    x = np.ascontiguousarray(np.asarray(inputs["x"], dtype=np.float32))
    readout_x = np.asarray(inputs["readout_x"], dtype=np.int32)
    W1 = np.asarray(inputs["W1"], dtype=np.float32)
    W2 = np.asarray(inputs["W2"], dtype=np.float32)
    Wh = np.asarray(inputs["Wh"], dtype=np.float32)
    Ws = np.asarray(inputs["Ws"], dtype=np.float32)
    Wc = np.asarray(inputs["Wc"], dtype=np.float32)
    b1 = np.asarray(inputs["b1"], dtype=np.float32)
    b2 = np.asarray(inputs["b2"], dtype=np.float32)
    bh = np.asarray(inputs["bh"], dtype=np.float32)
    bs = np.asarray(inputs["bs"], dtype=np.float32)
    bc = np.asarray(inputs["bc"], dtype=np.float32)

    # The kernel hardcodes the cyclic PAD/brick/comb routing. If the actual
    # readout pattern differs, permute x on the host so the device sees the
    # canonical layout (mirrors jnp.nonzero(..., size=ntok) semantics).
    ntok = TS_ * B
    rf = readout_x.reshape(-1)
    canonical = np.array_equal(
        readout_x, np.broadcast_to((np.arange(S, dtype=np.int32) % 3)[:, None], (S, B))
    )
    if not canonical:
        xf = x.reshape(S * B, C)
        xc = np.zeros_like(x).reshape(S * B, C)
        for name_idx in (1, 2):
            idx = np.nonzero(rf == name_idx)[0]
            if idx.shape[0] < ntok:
                idx = np.pad(idx, (0, ntok - idx.shape[0]))
            else:
                idx = idx[:ntok]
            tgt = (3 * (np.arange(ntok) // B) + name_idx) * B + (np.arange(ntok) % B)
            xc[tgt] = xf[idx]
        x = xc.reshape(S, B, C)

    # feature-major fp16 pack: [name, core, C, t, b] with brick first, then
    # block-major cols per core: [p, ni, block, k, tok] so each (name,
    # block) is one contiguous 4KB-line DMA
    xh = x.astype(np.float16).reshape(TS_, 3, NCORES, BL, C)
    xp = xh[:, 1:3].transpose(1, 2, 4, 0, 3)  # [name, core, C, t, b]

    Wsc = np.ascontiguousarray(np.concatenate([Ws, Wc], axis=1).astype(np.float16))
    W1h = np.ascontiguousarray(W1.astype(np.float16))
    W2h = np.ascontiguousarray(W2.astype(np.float16))
    Whh = np.ascontiguousarray(Wh.astype(np.float16))
    b12 = np.ascontiguousarray(
        np.concatenate([b1.reshape(KC, 128).T, b2.reshape(KC, 128).T], axis=1)
    )
    biasA = np.concatenate([bs, bc, bh])
    biasA_b = np.ascontiguousarray(np.broadcast_to(biasA, (128, A)))

    in_maps = []
    for c in range(NCORES):
        # brick ni=0 from readout name 1, comb ni=1 from name 2
        arr = xp[:, c].reshape(2, KC, 128, TS_, BL)  # [ni, k, p, t, b]
        segs = []
        for ni in range(2):
            for bi, nt in enumerate(SCHED):
                t0 = OFFS[bi] * TPB
                seg = arr[ni, :, :, t0 : t0 + nt * TPB, :]  # [k, p, tt, b]
                segs.append(
                    seg.transpose(1, 0, 2, 3).reshape(128, KC * nt * TT)
                )
        xt_core = np.ascontiguousarray(np.concatenate(segs, axis=1))
        in_maps.append(
            {
                "xt": xt_core,
                "w1": W1h,
                "w2": W2h,
                "wh": Whh,
                "wsc": Wsc,
                "b12": b12,
                "biasA": biasA_b,
            }
        )
    return in_maps


def _run(inputs, trace=False, trace_kwargs=None):
    nc = _build()
    in_maps = _prepare_inputs(inputs)
    res = run_bass_kernel_spmd(
        nc,
        in_maps,
        list(range(NCORES)),
        trace=trace,
        **(trace_kwargs or {}),
    )
    out = np.empty((TS_, B, A), dtype=np.float32)
    for c in range(NCORES):
        out[:, c * BL : (c + 1) * BL, :] = (
            res.results[c]["out"].reshape(TS_, BL, A).astype(np.float32)
        )
    return out, res


def kernel(**inputs) -> np.ndarray:
    out, _ = _run(inputs, trace=False)
    return out


if __name__ == "__main__":
    nc = _build()
    print("built OK")



# revision 54
# speedup vs baseline: 1.0179x; 1.0079x over previous
"""Trainium2 Bass kernel for nn_AutoDecoder (moe_routing).

Reference computation (per full input):
  x: [S=3072, B=32, C=512]; rows s%3==1 are "brick" tokens, s%3==2 are
  "combined" tokens (s%3==0 PAD rows are dead). For each (timestep, batch)
  pair:
    brick:  logits[0:80]    = x_brick @ [Ws|Wc]            (+ biases)
    comb:   h = relu(relu(x_comb @ W1 + b1) @ W2 + b2)
            logits[80:1000] = h @ Wh + bh
  out: [TS=1024, B=32, A=1000]

Strategy: data-parallel over batch (4 batch entries per core, 8 cores),
weights replicated. The host pre-packs x into feature-major fp16 tiles
(one [C, ntok] panel per readout name per core), so the device does no
transposes at all: per 512-token block it streams the 2-layer MLP
(fp16 weights, fp32 PSUM accumulation, relu+bias on ACT) and then the
head matmuls with the activations as stationary operands, producing
token-major logits in PSUM. The DVE adds the output bias while
downcasting to fp16, and fully contiguous DMAs write the fp16 logits
back (host upcasts to fp32).

Scheduling, driven by trace analysis: the kernel is PE-bound mid-run
(matmul issue at the measured peak rate: 35/183/206/216ns spacings),
so all slack lives at the edges. Startup is DMA-supply-bound (3 DMA
rings at ~115-130 GB/s each, ~2.4us spin-up, ~4 outstanding DMAs per
HW ring — a 5th issue stalls the issuing engine's FIFO, so the scalar
engine carries exactly the block-0 comb x issues and nothing else
before its ACTIVATEs). The first blocks' heads — the only consumers
of wh — are deferred 2-3 blocks and the PE fills in with MLP-only
work; a dummy-matmul warmup sized to bridge the ring spin-up keeps
the PE continuously busy so the HAM clock reaches 2.4GHz by ~11.5us
with no mid-run re-throttle; the first two blocks run k-outer so
matmuls stream with chunk arrivals, and the trailing 2-wide blocks
run k-outer to avoid ACT-gated matmul pacing. x is packed block-major
on the host so blocks>=2 load each name as ONE contiguous 4KB-line
DMA (the SW-DGE queue is descriptor-bound; strided 1KB lines defeat
its coalescer). Output tiles are written in two column halves on the
two hardware DMA queues; the very last tile computes its upper half
first into a free ps_h psum bank and quarter-splits its output DMAs
across both rings to shorten the drain tail.
"""
import sys

if "/opt/trn_rl_repo" not in sys.path:
    sys.path.append("/opt/trn_rl_repo")

import numpy as np

import concourse.bass as bass
from concourse import bacc
import concourse.mybir as mybir
import concourse.tile as tile
from concourse.bass import ts
from concourse.bass_utils import run_bass_kernel_spmd

F32 = mybir.dt.float32
F16 = mybir.dt.float16
RELU = mybir.ActivationFunctionType.Relu

# problem dims (hardcoded; kernel.py must be self-contained)
S, B, C = 3072, 32, 512
TS_ = S // 3                    # 1024 timesteps
NUM_SHAPES, NUM_COLORS, N_COMBINED = 64, 16, 920
NBRICK = NUM_SHAPES + NUM_COLORS  # 80
A = NBRICK + N_COMBINED           # 1000
NCORES = 8
BL = B // NCORES                  # 4 batch entries per core
NTOKC = TS_ * BL                  # 4096 tokens per name per core
TT = 128                          # tokens per tok-tile
TPB = TT // BL                    # 32 timesteps per tok-tile
KC = C // 128                     # 4 contraction chunks

# block schedule (tok-tiles per block): big first blocks bridge the startup
# DMA supply phase; small last blocks shorten the drain tail. Shared between
# the kernel build and the host-side x pack (the DRAM x layout is
# block-major so each block's per-name load is one contiguous DMA).
SCHED = [4] * 7 + [2, 2]
OFFS = [sum(SCHED[:i]) for i in range(len(SCHED))]
assert sum(SCHED) == TS_ // TPB

_BUILD_CACHE = {}


def _build():
    if "nc" in _BUILD_CACHE:
        return _BUILD_CACHE["nc"]
    nc = bacc.Bacc("TRN2", target_bir_lowering=False, debug=False)

    # feature-major fp16 x, block-major pack: 128 partition rows (feature
    # p within chunk); cols = [name ni][block bi][chunk k][token] so one
    # (name, block) load is a single contiguous DMA with 4KB lines — the
    # SW-DGE queue is descriptor-bound and the old 1KB strided lines
    # defeated its packet coalescer.
    xt_d = nc.declare_dram_parameter("xt", [128, 2 * KC * NTOKC], F16, isOutput=False)
    w1_d = nc.declare_dram_parameter("w1", [C, C], F16, isOutput=False)
    w2_d = nc.declare_dram_parameter("w2", [C, C], F16, isOutput=False)
    wh_d = nc.declare_dram_parameter("wh", [C, N_COMBINED], F16, isOutput=False)
    wsc_d = nc.declare_dram_parameter("wsc", [C, NBRICK], F16, isOutput=False)
    b12_d = nc.declare_dram_parameter("b12", [128, 2 * KC], F32, isOutput=False)
    bA_d = nc.declare_dram_parameter("biasA", [128, A], F32, isOutput=False)
    out_d = nc.declare_dram_parameter("out", [TS_ * BL, A], F16, isOutput=True)

    with tile.TileContext(nc) as tc:
        with (
            tc.tile_pool(name="const", bufs=1) as const,
            tc.tile_pool(name="xt", bufs=4) as xt_p,
            tc.tile_pool(name="h", bufs=4) as h_p,
            tc.tile_pool(name="osb", bufs=4) as o_p,
            tc.tile_pool(name="psh", bufs=4, space=bass.MemorySpace.PSUM) as ps_h,
            tc.tile_pool(name="psc", bufs=2, space=bass.MemorySpace.PSUM) as ps_c,
        ):
            # HAM warmup: dummy matmuls at t=0 (on a memset scratch, no DMA
            # dependency) sized to keep the PE continuously busy until the
            # first real weight/x chunks land (~2.3us after the queues spin
            # up), so the HAM clock ramp completes as early as possible and
            # the PE never goes idle before the first real matmul.
            warm_src = const.tile([128, 512], F16, tag="warm")
            nc.vector.memset(warm_src[:], 0.0)
            warm = ps_h.tile([128, 512], F32, tag="hps")
            for _ in range(4):
                nc.tensor.matmul(warm[:, 0:512], warm_src[:, 0:128], warm_src[:])
            for _ in range(10):
                nc.tensor.matmul(warm[:, 0:128], warm_src[:, 0:128], warm_src[:, 0:128])
            # pre-fire the one-time ACT activation-table load so the first
            # real relu doesn't pay ~1.3us for it
            warm_act = const.tile([128, 1], F32, tag="warmact")
            nc.scalar.activation(warm_act[0:1, 0:1], warm_src[0:1, 0:1], RELU)

            # Startup DMA placement (per-HW-queue ~115-130 GB/s for 1KB-line
            # tiles; ~2.4us ring spin-up from first issue to first packet;
            # SW-DGE gpsimd queue coalesces into 4KB packets; HW queues hold
            # only ~4 outstanding DMAs, so a 5th dma_start stalls at the
            # engine's FIFO head until a transfer completes). The scalar
            # engine issues ONLY the 4 block-0 comb x loads and then nothing
            # but ACTIVATEs + out-hi DMAs — any further DMA issue on scalar
            # queues ahead of the critical RELUs in its strict FIFO and
            # stalls layer 2 (a 2.4us PE gap in the baseline trace).
            #   sync:   w1 k0..k3, w2 k0..k3, wh k0..k3, then out-lo
            #   scalar: block-0 comb x (exactly 4 issues — a 5th DMA issue
            #           would stall at the FIFO head on ring backpressure
            #           and block every ACT behind it), then out-hi
            #   gpsimd: b1/b2 (tiny), wsc, block-1 x, block-0 brick x,
            #           biasA, then per-block x prefetch only
            w1_sb = []
            w2_sb = []
            for name, dram, out_list in (("w1", w1_d, w1_sb), ("w2", w2_d, w2_sb)):
                for k in range(KC):
                    t = const.tile([128, C], F16, tag=f"{name}_{k}")
                    out_list.append(t)
            for k in range(KC):
                nc.sync.dma_start(w1_sb[k][:], w1_d[ts(k, 128), :])
            b12_sb = const.tile([128, 2 * KC], F32, tag="b12")
            nc.gpsimd.dma_start(b12_sb[:], b12_d[:, :])
            wh_sb = []
            wsc_sb = []
            for k in range(KC):
                t = const.tile([128, NBRICK], F16, tag=f"wsc_{k}")
                nc.gpsimd.dma_start(t[:], wsc_d[ts(k, 128), :])
                wsc_sb.append(t)
            # w2 k0..k2 ride the SW queue ahead of block-1's x (land
            # ~11-13us) while k3 follows w1 on sync — layer 2 of block 0 is
            # then supplied early and depends as little as possible on the
            # sync ring, whose startup throughput jitters run to run
            for k in range(3):
                nc.gpsimd.dma_start(w2_sb[k][:], w2_d[ts(k, 128), :])

            def w1s(k, m):
                return w1_sb[k][:, ts(m, 128)]

            def w2s(k, m):
                return w2_sb[k][:, ts(m, 128)]

            def load_late_consts():
                # wh on sync behind w1/w2 (lands ~25.5us, needed at the
                # first finals flush ~27us); biasA DMAs are emitted on the
                # gpsimd SW queue after the early x loads
                for k in range(KC):
                    t = const.tile([128, N_COMBINED], F16, tag=f"wh_{k}")
                    nc.sync.dma_start(t[:], wh_d[ts(k, 128), :])
                    wh_sb.append(t)
                bA_sb = const.tile([128, A], F32, tag="biasA")
                return bA_sb

            # heads for block i, emitted during block i+1: token-major
            # logits straight into PSUM ([0:80] brick, [80:1000] comb),
            # DVE bias-add + fp16 downcast, contiguous DMA out.
            def finals(pb, hi_eng=None):
                # hi_eng: queue for the upper-half out DMA. During the
                # trailing 1-wide blocks the scalar FIFO is the ACT critical
                # chain, so those flushes route hi on sync; the very last
                # tile keeps scalar so the two final transfers parallelize.
                hi_eng = hi_eng or nc.scalar
                last = pb.get("last", False)
                for t in range(pb["nt"]):
                    t_last = last and t == pb["nt"] - 1
                    if t_last:
                        # final tile: its psum halves come from the ps_h
                        # pool (the MLP psums drained already) so the MMs
                        # don't wait for t0's pco to be released by the DVE;
                        # upper comb half first, so its psum stops early and
                        # its bias-add + DMA overlap the lower half's MMs.
                        pco_hi = ps_h.tile([128, 512], F32, tag="hps")
                        pco_lo = ps_h.tile([128, 512], F32, tag="hps")
                        ot = o_p.tile([128, A], F16, tag="osb")
                        r0 = (pb["t0"] + t * TPB) * BL
                        for k in range(KC):
                            nc.tensor.matmul(
                                pco_hi[:, 0 : A - 512],
                                pb["h2"][k][:, ts(t, 128)],
                                wh_sb[k][:, 512 - NBRICK : N_COMBINED],
                                start=(k == 0),
                                stop=(k == KC - 1),
                            )
                        nc.vector.tensor_add(
                            ot[:, 512:A], pco_hi[:, 0 : A - 512], bA_sb[:, 512:A]
                        )
                        # quarter-split out DMAs across both rings so the
                        # post-last-matmul transfer is ~64KB per ring, not
                        # one 128KB straggler on a single ring
                        nc.scalar.dma_start(out_d[r0 : r0 + TT, 512:768], ot[:, 512:768])
                        nc.sync.dma_start(out_d[r0 : r0 + TT, 768:A], ot[:, 768:A])
                        for k in range(KC):
                            nc.tensor.matmul(
                                pco_lo[:, 0:NBRICK],
                                pb["xtb"][k][:, ts(t, 128)],
                                wsc_sb[k][:],
                                start=(k == 0),
                                stop=(k == KC - 1),
                            )
                        for k in range(KC):
                            nc.tensor.matmul(
                                pco_lo[:, NBRICK:512],
                                pb["h2"][k][:, ts(t, 128)],
                                wh_sb[k][:, 0 : 512 - NBRICK],
                                start=(k == 0),
                                stop=(k == KC - 1),
                            )
                        # lo half in quarter adds so each ring's final DMA
                        # issues as soon as its quarter is biased
                        nc.vector.tensor_add(
                            ot[:, 0:256], pco_lo[:, 0:256], bA_sb[:, 0:256]
                        )
                        nc.scalar.dma_start(out_d[r0 : r0 + TT, 0:256], ot[:, 0:256])
                        nc.vector.tensor_add(
                            ot[:, 256:512], pco_lo[:, 256:512], bA_sb[:, 256:512]
                        )
                        nc.sync.dma_start(out_d[r0 : r0 + TT, 256:512], ot[:, 256:512])
                        continue
                    pco = ps_c.tile([128, 1024], F32, tag="combo")
                    for k in range(KC):
                        lhs = pb["h2"][k][:, ts(t, 128)]
                        nc.tensor.matmul(
                            pco[:, NBRICK:512],
                            lhs,
                            wh_sb[k][:, 0 : 512 - NBRICK],
                            start=(k == 0),
                            stop=(k == KC - 1),
                        )
                        nc.tensor.matmul(
                            pco[:, 512:A],
                            lhs,
                            wh_sb[k][:, 512 - NBRICK : N_COMBINED],
                            start=(k == 0),
                            stop=(k == KC - 1),
                        )
                    for k in range(KC):
                        nc.tensor.matmul(
                            pco[:, 0:NBRICK],
                            pb["xtb"][k][:, ts(t, 128)],
                            wsc_sb[k][:],
                            start=(k == 0),
                            stop=(k == KC - 1),
                        )
                    # bias-add + fp16 downcast in two column halves so the
                    # first half's DMA fires while the DVE does the second
                    # half; halves go to different queues
                    ot = o_p.tile([128, A], F16, tag="osb")
                    r0 = (pb["t0"] + t * TPB) * BL
                    nc.vector.tensor_add(ot[:, 0:512], pco[:, 0:512], bA_sb[:, 0:512])
                    nc.sync.dma_start(out_d[r0 : r0 + TT, 0:512], ot[:, 0:512])
                    nc.vector.tensor_add(ot[:, 512:A], pco[:, 512:A], bA_sb[:, 512:A])
                    hi_eng.dma_start(out_d[r0 : r0 + TT, 512:A], ot[:, 512:A])

            # ---- main loop over blocks of tok-tiles (128 tokens each) ----
            sched = SCHED
            offs = OFFS
            # heads deferral: the first ~25us are DMA-limited (weights still
            # streaming in at ~60-80 GB/s per queue), so the first blocks'
            # heads — which need wh — are pushed back 1-2 extra blocks; the
            # PE fills that time with MLP work that only needs w1/w2/x. The
            # backlog is worked off two-per-block once wh has landed.
            nblk = len(sched)
            flush_n = {2: 1, 3: 2}
            flush_n.update({i: 1 for i in range(4, nblk)})

            # feature-major fp16 x loads, issued one block AHEAD so the
            # queue has a full block period to deliver. Block 0's comb x
            # goes on the scalar HW queue (parallel with w1 on sync) and
            # its brick x on gpsimd; later blocks use the gpsimd
            # software-DGE queue entirely. Blocks 0/1 split per k-chunk
            # (compute streams with chunk arrivals; block 0's first chunk
            # is halved again for the earliest possible first matmul);
            # later blocks load each name as ONE contiguous 4KB-line DMA.
            def issue_name_loads(bi, ni, eng):
                nt = sched[bi]
                W_ = nt * TT
                c0 = ni * (KC * NTOKC) + offs[bi] * TT * KC
                tl = xt_p.tile([128, KC * W_], F16, tag=f"xt{ni}")
                if bi == 0 and ni == 1:
                    eng.dma_start(tl[:, 0:W_], xt_d[:, c0 : c0 + W_])
                    eng.dma_start(tl[:, W_ : 2 * W_], xt_d[:, c0 + W_ : c0 + 2 * W_])
                    eng.dma_start(
                        tl[:, 2 * W_ : 4 * W_], xt_d[:, c0 + 2 * W_ : c0 + 4 * W_]
                    )
                elif bi == 1:
                    for k in range(KC):
                        eng.dma_start(
                            tl[:, k * W_ : (k + 1) * W_],
                            xt_d[:, c0 + k * W_ : c0 + (k + 1) * W_],
                        )
                else:
                    eng.dma_start(tl[:], xt_d[:, c0 : c0 + KC * W_])
                return [tl[:, k * W_ : (k + 1) * W_] for k in range(KC)]

            def issue_loads(bi):
                return (
                    issue_name_loads(bi, 1, nc.gpsimd),
                    issue_name_loads(bi, 0, nc.gpsimd),
                )

            xtc0 = issue_name_loads(0, 1, nc.scalar)
            for k in range(3, KC):
                nc.sync.dma_start(w2_sb[k][:], w2_d[ts(k, 128), :])
            bA_sb = load_late_consts()
            issued = {1: issue_loads(1)}
            # block 0's brick x goes behind block 1's x on the SW queue —
            # it is not read until the deferred heads flush
            issued[0] = (xtc0, issue_name_loads(0, 0, nc.gpsimd))
            # biasA rides the deep SW-DGE ring behind the early x loads
            # (lands ~22us, first bias-add needs it ~27us); the HW rings
            # would either stall ACTs (scalar) or push wh too late (sync)
            nc.gpsimd.dma_start(bA_sb[:, 0:512], bA_d[:, 0:512])
            nc.gpsimd.dma_start(bA_sb[:, 512:A], bA_d[:, 512:A])
            pendings = []
            for bi, nt in enumerate(sched):
                ti0 = offs[bi]
                t0 = ti0 * TPB
                W_ = nt * TT  # tokens per name in this block
                xtc, xtb = issued.pop(bi)
                if bi + 1 < len(sched) and bi + 1 not in issued:
                    issued[bi + 1] = issue_loads(bi + 1)

                # deferred heads of earlier blocks (see note above)
                for _ in range(flush_n.get(bi, 0)):
                    finals(
                        pendings.pop(0),
                        hi_eng=nc.sync if bi >= nblk - 2 else nc.scalar,
                    )

                # comb MLP: hT[m] = relu(W[:,m-chunk].T @ inT + b). The first
                # two blocks run k-outer so matmuls stream with the arriving
                # weight/x DMA chunks; the trailing 1-wide blocks run k-outer
                # too (m-outer at W=128 is ACT-gated: ~190ns/MM instead of
                # 56); mid-run 4-wide blocks run m-outer so the ACT engine
                # drains each m-psum while the next accumulates.
                def layer(ws, inp, btag, hname):
                    hs_list = []
                    if bi < 2 or bi >= nblk - 2:
                        phs = []
                        for _m in range(KC):
                            ph = ps_h.tile([128, W_], F32, tag="hps")
                            phs.append(ph)
                        # NOTE: do not split a k-stage into column sub-ranges
                        # of one psum bank — a bank supports only ONE open
                        # accumulation group at a time, and a second start=
                        # True group in the same bank corrupts the first.
                        for k in range(KC):
                            for m in range(KC):
                                nc.tensor.matmul(
                                    phs[m][:],
                                    ws(k, m),
                                    inp[k][:],
                                    start=(k == 0),
                                    stop=(k == KC - 1),
                                )
                        for m in range(KC):
                            hs = h_p.tile([128, W_], F16, tag=f"{hname}_{m}")
                            nc.scalar.activation(
                                hs[:],
                                phs[m][:],
                                RELU,
                                bias=b12_sb[:, btag + m : btag + m + 1],
                                scale=1.0,
                            )
                            hs_list.append(hs)
                    else:
                        for m in range(KC):
                            ph = ps_h.tile([128, W_], F32, tag="hps")
                            for k in range(KC):
                                nc.tensor.matmul(
                                    ph[:],
                                    ws(k, m),
                                    inp[k][:],
                                    start=(k == 0),
                                    stop=(k == KC - 1),
                                )
                            hs = h_p.tile([128, W_], F16, tag=f"{hname}_{m}")
                            nc.scalar.activation(
                                hs[:],
                                ph[:],
                                RELU,
                                bias=b12_sb[:, btag + m : btag + m + 1],
                                scale=1.0,
                            )
                            hs_list.append(hs)
                    return hs_list

                h1 = layer(w1s, xtc, 0, "h1")
                h2 = layer(w2s, h1, KC, "h2")

                pendings.append({"h2": h2, "xtb": xtb, "t0": t0, "nt": nt})
            assert len(pendings) == 1
            pendings[0]["last"] = True
            # keep-warm: the out rings have been idle for ~5us by now; a
            # tiny read on each re-spins them so the final out transfers
            # don't pay the ring restart latency
            kw = const.tile([1, C], F16, tag="kw")
            nc.sync.dma_start(kw[0:1, 0:256], w1_d[0:1, 0:256])
            nc.scalar.dma_start(kw[0:1, 256:512], w1_d[0:1, 256:512])
            finals(pendings[0])

    nc.compile()
    _BUILD_CACHE["nc"] = nc
    return nc


def _prepare_inputs(inputs):
    """Host-side prep: validate/normalize routing, shard over batch,
    pre-pack x feature-major fp16, replicate weights. Returns in_maps
    for the 8 cores."""
    x = np.ascontiguousarray(np.asarray(inputs["x"], dtype=np.float32))
    readout_x = np.asarray(inputs["readout_x"], dtype=np.int32)
    W1 = np.asarray(inputs["W1"], dtype=np.float32)
    W2 = np.asarray(inputs["W2"], dtype=np.float32)
    Wh = np.asarray(inputs["Wh"], dtype=np.float32)
    Ws = np.asarray(inputs["Ws"], dtype=np.float32)
    Wc = np.asarray(inputs["Wc"], dtype=np.float32)
    b1 = np.asarray(inputs["b1"], dtype=np.float32)
    b2 = np.asarray(inputs["b2"], dtype=np.float32)
    bh = np.asarray(inputs["bh"], dtype=np.float32)
    bs = np.asarray(inputs["bs"], dtype=np.float32)
    bc = np.asarray(inputs["bc"], dtype=np.float32)

    # The kernel hardcodes the cyclic PAD/brick/comb routing. If the actual
    # readout pattern differs, permute x on the host so the device sees the
    # canonical layout (mirrors jnp.nonzero(..., size=ntok) semantics).
    ntok = TS_ * B
    rf = readout_x.reshape(-1)
    canonical = np.array_equal(
        readout_x, np.broadcast_to((np.arange(S, dtype=np.int32) % 3)[:, None], (S, B))
    )
    if not canonical:
        xf = x.reshape(S * B, C)
        xc = np.zeros_like(x).reshape(S * B, C)
        for name_idx in (1, 2):
            idx = np.nonzero(rf == name_idx)[0]
            if idx.shape[0] < ntok:
                idx = np.pad(idx, (0, ntok - idx.shape[0]))
            else:
                idx = idx[:ntok]
            tgt = (3 * (np.arange(ntok) // B) + name_idx) * B + (np.arange(ntok) % B)
            xc[tgt] = xf[idx]
        x = xc.reshape(S, B, C)

    # feature-major fp16 pack: [name, core, C, t, b] with brick first, then
    # block-major cols per core: [p, ni, block, k, tok] so each (name,
    # block) is one contiguous 4KB-line DMA
    xh = x.astype(np.float16).reshape(TS_, 3, NCORES, BL, C)
    xp = xh[:, 1:3].transpose(1, 2, 4, 0, 3)  # [name, core, C, t, b]

    Wsc = np.ascontiguousarray(np.concatenate([Ws, Wc], axis=1).astype(np.float16))
    W1h = np.ascontiguousarray(W1.astype(np.float16))
    W2h = np.ascontiguousarray(W2.astype(np.float16))
    Whh = np.ascontiguousarray(Wh.astype(np.float16))
    b12 = np.ascontiguousarray(
        np.concatenate([b1.reshape(KC, 128).T, b2.reshape(KC, 128).T], axis=1)
    )
    biasA = np.concatenate([bs, bc, bh])
    biasA_b = np.ascontiguousarray(np.broadcast_to(biasA, (128, A)))

    in_maps = []
    for c in range(NCORES):
        # brick ni=0 from readout name 1, comb ni=1 from name 2
        arr = xp[:, c].reshape(2, KC, 128, TS_, BL)  # [ni, k, p, t, b]
        segs = []
        for ni in range(2):
            for bi, nt in enumerate(SCHED):
                t0 = OFFS[bi] * TPB
                seg = arr[ni, :, :, t0 : t0 + nt * TPB, :]  # [k, p, tt, b]
                segs.append(
                    seg.transpose(1, 0, 2, 3).reshape(128, KC * nt * TT)
                )
        xt_core = np.ascontiguousarray(np.concatenate(segs, axis=1))
        in_maps.append(
            {
                "xt": xt_core,
                "w1": W1h,
                "w2": W2h,
                "wh": Whh,
                "wsc": Wsc,
                "b12": b12,
                "biasA": biasA_b,
            }
        )
    return in_maps


def _run(inputs, trace=False, trace_kwargs=None):
    nc = _build()
    in_maps = _prepare_inputs(inputs)
    res = run_bass_kernel_spmd(
        nc,
        in_maps,
        list(range(NCORES)),
        trace=trace,
        **(trace_kwargs or {}),
    )
    out = np.empty((TS_, B, A), dtype=np.float32)
    for c in range(NCORES):
        out[:, c * BL : (c + 1) * BL, :] = (
            res.results[c]["out"].reshape(TS_, BL, A).astype(np.float32)
        )
    return out, res


def kernel(**inputs) -> np.ndarray:
    out, _ = _run(inputs, trace=False)
    return out


if __name__ == "__main__":
    nc = _build()
    print("built OK")



# revision 57
# speedup vs baseline: 1.0203x; 1.0024x over previous
"""Trainium2 Bass kernel for nn_AutoDecoder (moe_routing).

Reference computation (per full input):
  x: [S=3072, B=32, C=512]; rows s%3==1 are "brick" tokens, s%3==2 are
  "combined" tokens (s%3==0 PAD rows are dead). For each (timestep, batch)
  pair:
    brick:  logits[0:80]    = x_brick @ [Ws|Wc]            (+ biases)
    comb:   h = relu(relu(x_comb @ W1 + b1) @ W2 + b2)
            logits[80:1000] = h @ Wh + bh
  out: [TS=1024, B=32, A=1000]

Strategy: data-parallel over batch (4 batch entries per core, 8 cores),
weights replicated. The host pre-packs x into feature-major fp16 tiles
(one [C, ntok] panel per readout name per core), so the device does no
transposes at all: per 512-token block it streams the 2-layer MLP
(fp16 weights, fp32 PSUM accumulation, relu+bias on ACT) and then the
head matmuls with the activations as stationary operands, producing
token-major logits in PSUM. The DVE adds the output bias while
downcasting to fp16, and fully contiguous DMAs write the fp16 logits
back (host upcasts to fp32).

Scheduling, driven by trace analysis: the kernel is PE-bound mid-run
(matmul issue at the measured peak rate: 35/183/206/216ns spacings),
so all slack lives at the edges. Startup is DMA-supply-bound (3 DMA
rings at ~115-130 GB/s each, ~2.4us spin-up, ~4 outstanding DMAs per
HW ring — a 5th issue stalls the issuing engine's FIFO, so the scalar
engine carries exactly the block-0 comb x issues and nothing else
before its ACTIVATEs). The first blocks' heads — the only consumers
of wh — are deferred 2-3 blocks and the PE fills in with MLP-only
work; a dummy-matmul warmup sized to bridge the ring spin-up keeps
the PE continuously busy so the HAM clock reaches 2.4GHz by ~11.5us
with no mid-run re-throttle; the first two blocks run k-outer so
matmuls stream with chunk arrivals, and the trailing 2-wide blocks
run k-outer to avoid ACT-gated matmul pacing. x is packed block-major
on the host so blocks>=2 load each name as ONE contiguous 4KB-line
DMA (the SW-DGE queue is descriptor-bound; strided 1KB lines defeat
its coalescer). Output tiles are written in two column halves on the
two hardware DMA queues; the very last tile computes its upper half
first into a free ps_h psum bank and quarter-splits its output DMAs
across both rings to shorten the drain tail.
"""
import sys

if "/opt/trn_rl_repo" not in sys.path:
    sys.path.append("/opt/trn_rl_repo")

import numpy as np

import concourse.bass as bass
from concourse import bacc
import concourse.mybir as mybir
import concourse.tile as tile
from concourse.bass import ts
from concourse.bass_utils import run_bass_kernel_spmd

F32 = mybir.dt.float32
F16 = mybir.dt.float16
RELU = mybir.ActivationFunctionType.Relu

# problem dims (hardcoded; kernel.py must be self-contained)
S, B, C = 3072, 32, 512
TS_ = S // 3                    # 1024 timesteps
NUM_SHAPES, NUM_COLORS, N_COMBINED = 64, 16, 920
NBRICK = NUM_SHAPES + NUM_COLORS  # 80
A = NBRICK + N_COMBINED           # 1000
NCORES = 8
BL = B // NCORES                  # 4 batch entries per core
NTOKC = TS_ * BL                  # 4096 tokens per name per core
TT = 128                          # tokens per tok-tile
TPB = TT // BL                    # 32 timesteps per tok-tile
KC = C // 128                     # 4 contraction chunks

# block schedule (tok-tiles per block): big first blocks bridge the startup
# DMA supply phase; small last blocks shorten the drain tail. Shared between
# the kernel build and the host-side x pack (the DRAM x layout is
# block-major so each block's per-name load is one contiguous DMA).
SCHED = [4] * 7 + [2, 2]
OFFS = [sum(SCHED[:i]) for i in range(len(SCHED))]
assert sum(SCHED) == TS_ // TPB

_BUILD_CACHE = {}


def _build():
    if "nc" in _BUILD_CACHE:
        return _BUILD_CACHE["nc"]
    nc = bacc.Bacc("TRN2", target_bir_lowering=False, debug=False)

    # feature-major fp16 x, block-major pack: 128 partition rows (feature
    # p within chunk); cols = [name ni][block bi][chunk k][token] so one
    # (name, block) load is a single contiguous DMA with 4KB lines — the
    # SW-DGE queue is descriptor-bound and the old 1KB strided lines
    # defeated its packet coalescer.
    xt_d = nc.declare_dram_parameter("xt", [128, 2 * KC * NTOKC], F16, isOutput=False)
    w1_d = nc.declare_dram_parameter("w1", [C, C], F16, isOutput=False)
    w2_d = nc.declare_dram_parameter("w2", [C, C], F16, isOutput=False)
    wh_d = nc.declare_dram_parameter("wh", [C, N_COMBINED], F16, isOutput=False)
    wsc_d = nc.declare_dram_parameter("wsc", [C, NBRICK], F16, isOutput=False)
    b12_d = nc.declare_dram_parameter("b12", [128, 2 * KC], F32, isOutput=False)
    bA_d = nc.declare_dram_parameter("biasA", [128, A], F32, isOutput=False)
    out_d = nc.declare_dram_parameter("out", [TS_ * BL, A], F16, isOutput=True)

    with tile.TileContext(nc) as tc:
        with (
            tc.tile_pool(name="const", bufs=1) as const,
            tc.tile_pool(name="xt", bufs=4) as xt_p,
            tc.tile_pool(name="h", bufs=4) as h_p,
            tc.tile_pool(name="osb", bufs=4) as o_p,
            tc.tile_pool(name="psh", bufs=4, space=bass.MemorySpace.PSUM) as ps_h,
            tc.tile_pool(name="psc", bufs=2, space=bass.MemorySpace.PSUM) as ps_c,
        ):
            # HAM warmup: dummy matmuls at t=0 (on a memset scratch, no DMA
            # dependency) sized to keep the PE continuously busy until the
            # first real weight/x chunks land (~2.3us after the queues spin
            # up), so the HAM clock ramp completes as early as possible and
            # the PE never goes idle before the first real matmul.
            warm_src = const.tile([128, 512], F16, tag="warm")
            nc.vector.memset(warm_src[:], 0.0)
            warm = ps_h.tile([128, 512], F32, tag="hps")
            for _ in range(4):
                nc.tensor.matmul(warm[:, 0:512], warm_src[:, 0:128], warm_src[:])
            for _ in range(10):
                nc.tensor.matmul(warm[:, 0:128], warm_src[:, 0:128], warm_src[:, 0:128])
            # pre-fire the one-time ACT activation-table load so the first
            # real relu doesn't pay ~1.3us for it
            warm_act = const.tile([128, 1], F32, tag="warmact")
            nc.scalar.activation(warm_act[0:1, 0:1], warm_src[0:1, 0:1], RELU)

            # Startup DMA placement (per-HW-queue ~115-130 GB/s for 1KB-line
            # tiles; ~2.4us ring spin-up from first issue to first packet;
            # SW-DGE gpsimd queue coalesces into 4KB packets; HW queues hold
            # only ~4 outstanding DMAs, so a 5th dma_start stalls at the
            # engine's FIFO head until a transfer completes). The scalar
            # engine issues ONLY the 4 block-0 comb x loads and then nothing
            # but ACTIVATEs + out-hi DMAs — any further DMA issue on scalar
            # queues ahead of the critical RELUs in its strict FIFO and
            # stalls layer 2 (a 2.4us PE gap in the baseline trace).
            #   sync:   w1 k0..k3, w2 k0..k3, wh k0..k3, then out-lo
            #   scalar: block-0 comb x (exactly 4 issues — a 5th DMA issue
            #           would stall at the FIFO head on ring backpressure
            #           and block every ACT behind it), then out-hi
            #   gpsimd: b1/b2 (tiny), wsc, block-1 x, block-0 brick x,
            #           biasA, then per-block x prefetch only
            w1_sb = []
            w2_sb = []
            for name, dram, out_list in (("w1", w1_d, w1_sb), ("w2", w2_d, w2_sb)):
                for k in range(KC):
                    t = const.tile([128, C], F16, tag=f"{name}_{k}")
                    out_list.append(t)
            for k in range(KC):
                nc.sync.dma_start(w1_sb[k][:], w1_d[ts(k, 128), :])
            b12_sb = const.tile([128, 2 * KC], F32, tag="b12")
            nc.gpsimd.dma_start(b12_sb[:], b12_d[:, :])
            # w2 k0..k2 ride the SW queue ahead of block-1's x (land
            # ~11-13us) while k3 follows w1 on sync — layer 2 of block 0 is
            # then supplied early and depends as little as possible on the
            # sync ring, whose startup throughput jitters run to run. wsc
            # follows (not needed until the first flush ~27us).
            for k in range(3):
                nc.gpsimd.dma_start(w2_sb[k][:], w2_d[ts(k, 128), :])
            wh_sb = []
            wsc_sb = []
            for k in range(KC):
                t = const.tile([128, NBRICK], F16, tag=f"wsc_{k}")
                nc.gpsimd.dma_start(t[:], wsc_d[ts(k, 128), :])
                wsc_sb.append(t)

            def w1s(k, m):
                return w1_sb[k][:, ts(m, 128)]

            def w2s(k, m):
                return w2_sb[k][:, ts(m, 128)]

            def load_late_consts():
                # wh on sync behind w1/w2 (lands ~25.5us, needed at the
                # first finals flush ~27us); biasA DMAs are emitted on the
                # gpsimd SW queue after the early x loads
                for k in range(KC):
                    t = const.tile([128, N_COMBINED], F16, tag=f"wh_{k}")
                    nc.sync.dma_start(t[:], wh_d[ts(k, 128), :])
                    wh_sb.append(t)
                bA_sb = const.tile([128, A], F32, tag="biasA")
                return bA_sb

            # heads for block i, emitted during block i+1: token-major
            # logits straight into PSUM ([0:80] brick, [80:1000] comb),
            # DVE bias-add + fp16 downcast, contiguous DMA out.
            def finals(pb, hi_eng=None):
                # hi_eng: queue for the upper-half out DMA. During the
                # trailing 1-wide blocks the scalar FIFO is the ACT critical
                # chain, so those flushes route hi on sync; the very last
                # tile keeps scalar so the two final transfers parallelize.
                hi_eng = hi_eng or nc.scalar
                last = pb.get("last", False)
                for t in range(pb["nt"]):
                    t_last = last and t == pb["nt"] - 1
                    if t_last:
                        # final tile: its psum halves come from the ps_h
                        # pool (the MLP psums drained already) so the MMs
                        # don't wait for t0's pco to be released by the DVE;
                        # upper comb half first, so its psum stops early and
                        # its bias-add + DMA overlap the lower half's MMs.
                        pco_hi = ps_h.tile([128, 512], F32, tag="hps")
                        pco_lo = ps_h.tile([128, 512], F32, tag="hps")
                        ot = o_p.tile([128, A], F16, tag="osb")
                        r0 = (pb["t0"] + t * TPB) * BL
                        for k in range(KC):
                            nc.tensor.matmul(
                                pco_hi[:, 0 : A - 512],
                                pb["h2"][k][:, ts(t, 128)],
                                wh_sb[k][:, 512 - NBRICK : N_COMBINED],
                                start=(k == 0),
                                stop=(k == KC - 1),
                            )
                        nc.vector.tensor_add(
                            ot[:, 512:A], pco_hi[:, 0 : A - 512], bA_sb[:, 512:A]
                        )
                        # quarter-split out DMAs across both rings so the
                        # post-last-matmul transfer is ~64KB per ring, not
                        # one 128KB straggler on a single ring
                        nc.scalar.dma_start(out_d[r0 : r0 + TT, 512:768], ot[:, 512:768])
                        nc.sync.dma_start(out_d[r0 : r0 + TT, 768:A], ot[:, 768:A])
                        for k in range(KC):
                            nc.tensor.matmul(
                                pco_lo[:, 0:NBRICK],
                                pb["xtb"][k][:, ts(t, 128)],
                                wsc_sb[k][:],
                                start=(k == 0),
                                stop=(k == KC - 1),
                            )
                        for k in range(KC):
                            nc.tensor.matmul(
                                pco_lo[:, NBRICK:512],
                                pb["h2"][k][:, ts(t, 128)],
                                wh_sb[k][:, 0 : 512 - NBRICK],
                                start=(k == 0),
                                stop=(k == KC - 1),
                            )
                        # lo half in quarter adds so each ring's final DMA
                        # issues as soon as its quarter is biased
                        nc.vector.tensor_add(
                            ot[:, 0:256], pco_lo[:, 0:256], bA_sb[:, 0:256]
                        )
                        nc.scalar.dma_start(out_d[r0 : r0 + TT, 0:256], ot[:, 0:256])
                        nc.vector.tensor_add(
                            ot[:, 256:512], pco_lo[:, 256:512], bA_sb[:, 256:512]
                        )
                        nc.sync.dma_start(out_d[r0 : r0 + TT, 256:512], ot[:, 256:512])
                        continue
                    pco = ps_c.tile([128, 1024], F32, tag="combo")
                    for k in range(KC):
                        lhs = pb["h2"][k][:, ts(t, 128)]
                        nc.tensor.matmul(
                            pco[:, NBRICK:512],
                            lhs,
                            wh_sb[k][:, 0 : 512 - NBRICK],
                            start=(k == 0),
                            stop=(k == KC - 1),
                        )
                        nc.tensor.matmul(
                            pco[:, 512:A],
                            lhs,
                            wh_sb[k][:, 512 - NBRICK : N_COMBINED],
                            start=(k == 0),
                            stop=(k == KC - 1),
                        )
                    for k in range(KC):
                        nc.tensor.matmul(
                            pco[:, 0:NBRICK],
                            pb["xtb"][k][:, ts(t, 128)],
                            wsc_sb[k][:],
                            start=(k == 0),
                            stop=(k == KC - 1),
                        )
                    # bias-add + fp16 downcast in two column halves so the
                    # first half's DMA fires while the DVE does the second
                    # half; halves go to different queues
                    ot = o_p.tile([128, A], F16, tag="osb")
                    r0 = (pb["t0"] + t * TPB) * BL
                    nc.vector.tensor_add(ot[:, 0:512], pco[:, 0:512], bA_sb[:, 0:512])
                    nc.sync.dma_start(out_d[r0 : r0 + TT, 0:512], ot[:, 0:512])
                    nc.vector.tensor_add(ot[:, 512:A], pco[:, 512:A], bA_sb[:, 512:A])
                    hi_eng.dma_start(out_d[r0 : r0 + TT, 512:A], ot[:, 512:A])

            # ---- main loop over blocks of tok-tiles (128 tokens each) ----
            sched = SCHED
            offs = OFFS
            # heads deferral: the first ~25us are DMA-limited (weights still
            # streaming in at ~60-80 GB/s per queue), so the first blocks'
            # heads — which need wh — are pushed back 1-2 extra blocks; the
            # PE fills that time with MLP work that only needs w1/w2/x. The
            # backlog is worked off two-per-block once wh has landed.
            nblk = len(sched)
            flush_n = {2: 1, 3: 2}
            flush_n.update({i: 1 for i in range(4, nblk)})

            # feature-major fp16 x loads, issued one block AHEAD so the
            # queue has a full block period to deliver. Block 0's comb x
            # goes on the scalar HW queue (parallel with w1 on sync) and
            # its brick x on gpsimd; later blocks use the gpsimd
            # software-DGE queue entirely. Blocks 0/1 split per k-chunk
            # (compute streams with chunk arrivals; block 0's first chunk
            # is halved again for the earliest possible first matmul);
            # later blocks load each name as ONE contiguous 4KB-line DMA.
            def issue_name_loads(bi, ni, eng):
                nt = sched[bi]
                W_ = nt * TT
                c0 = ni * (KC * NTOKC) + offs[bi] * TT * KC
                tl = xt_p.tile([128, KC * W_], F16, tag=f"xt{ni}")
                if bi == 0 and ni == 1:
                    eng.dma_start(tl[:, 0:W_], xt_d[:, c0 : c0 + W_])
                    eng.dma_start(tl[:, W_ : 2 * W_], xt_d[:, c0 + W_ : c0 + 2 * W_])
                    eng.dma_start(
                        tl[:, 2 * W_ : 4 * W_], xt_d[:, c0 + 2 * W_ : c0 + 4 * W_]
                    )
                elif bi == 1:
                    for k in range(KC):
                        eng.dma_start(
                            tl[:, k * W_ : (k + 1) * W_],
                            xt_d[:, c0 + k * W_ : c0 + (k + 1) * W_],
                        )
                else:
                    eng.dma_start(tl[:], xt_d[:, c0 : c0 + KC * W_])
                return [tl[:, k * W_ : (k + 1) * W_] for k in range(KC)]

            def issue_loads(bi):
                return (
                    issue_name_loads(bi, 1, nc.gpsimd),
                    issue_name_loads(bi, 0, nc.gpsimd),
                )

            xtc0 = issue_name_loads(0, 1, nc.scalar)
            for k in range(3, KC):
                nc.sync.dma_start(w2_sb[k][:], w2_d[ts(k, 128), :])
            bA_sb = load_late_consts()
            issued = {1: issue_loads(1)}
            # block 0's brick x goes behind block 1's x on the SW queue —
            # it is not read until the deferred heads flush
            issued[0] = (xtc0, issue_name_loads(0, 0, nc.gpsimd))
            # biasA rides the deep SW-DGE ring behind the early x loads
            # (lands ~22us, first bias-add needs it ~27us); the HW rings
            # would either stall ACTs (scalar) or push wh too late (sync)
            nc.gpsimd.dma_start(bA_sb[:, 0:512], bA_d[:, 0:512])
            nc.gpsimd.dma_start(bA_sb[:, 512:A], bA_d[:, 512:A])
            pendings = []
            for bi, nt in enumerate(sched):
                ti0 = offs[bi]
                t0 = ti0 * TPB
                W_ = nt * TT  # tokens per name in this block
                xtc, xtb = issued.pop(bi)
                if bi + 1 < len(sched) and bi + 1 not in issued:
                    issued[bi + 1] = issue_loads(bi + 1)

                # deferred heads of earlier blocks (see note above); for
                # the trailing blocks they are emitted BETWEEN the two MLP
                # layers (below) so the PE covers the h1 ACT latency at the
                # layer boundary instead of idling ~290ns per boundary
                flushes = [pendings.pop(0) for _ in range(flush_n.get(bi, 0))]
                late = bi >= nblk - 2
                if not late:
                    for pb in flushes:
                        finals(pb)

                # comb MLP: hT[m] = relu(W[:,m-chunk].T @ inT + b). The first
                # two blocks run k-outer so matmuls stream with the arriving
                # weight/x DMA chunks; the trailing 1-wide blocks run k-outer
                # too (m-outer at W=128 is ACT-gated: ~190ns/MM instead of
                # 56); mid-run 4-wide blocks run m-outer so the ACT engine
                # drains each m-psum while the next accumulates.
                def layer(ws, inp, btag, hname):
                    hs_list = []
                    if bi < 2 or bi >= nblk - 2:
                        phs = []
                        for _m in range(KC):
                            ph = ps_h.tile([128, W_], F32, tag="hps")
                            phs.append(ph)
                        # NOTE: do not split a k-stage into column sub-ranges
                        # of one psum bank — a bank supports only ONE open
                        # accumulation group at a time, and a second start=
                        # True group in the same bank corrupts the first.
                        for k in range(KC):
                            for m in range(KC):
                                nc.tensor.matmul(
                                    phs[m][:],
                                    ws(k, m),
                                    inp[k][:],
                                    start=(k == 0),
                                    stop=(k == KC - 1),
                                )
                        for m in range(KC):
                            hs = h_p.tile([128, W_], F16, tag=f"{hname}_{m}")
                            nc.scalar.activation(
                                hs[:],
                                phs[m][:],
                                RELU,
                                bias=b12_sb[:, btag + m : btag + m + 1],
                                scale=1.0,
                            )
                            hs_list.append(hs)
                    else:
                        for m in range(KC):
                            ph = ps_h.tile([128, W_], F32, tag="hps")
                            for k in range(KC):
                                nc.tensor.matmul(
                                    ph[:],
                                    ws(k, m),
                                    inp[k][:],
                                    start=(k == 0),
                                    stop=(k == KC - 1),
                                )
                            hs = h_p.tile([128, W_], F16, tag=f"{hname}_{m}")
                            nc.scalar.activation(
                                hs[:],
                                ph[:],
                                RELU,
                                bias=b12_sb[:, btag + m : btag + m + 1],
                                scale=1.0,
                            )
                            hs_list.append(hs)
                    return hs_list

                h1 = layer(w1s, xtc, 0, "h1")
                if late:
                    for pb in flushes:
                        finals(pb, hi_eng=nc.sync)
                h2 = layer(w2s, h1, KC, "h2")

                pendings.append({"h2": h2, "xtb": xtb, "t0": t0, "nt": nt})
            assert len(pendings) == 1
            pendings[0]["last"] = True
            # keep-warm: the out rings have been idle for ~5us by now; a
            # tiny read on each re-spins them so the final out transfers
            # don't pay the ring restart latency
            kw = const.tile([1, C], F16, tag="kw")
            nc.sync.dma_start(kw[0:1, 0:256], w1_d[0:1, 0:256])
            nc.scalar.dma_start(kw[0:1, 256:512], w1_d[0:1, 256:512])
            finals(pendings[0])

    nc.compile()
    _BUILD_CACHE["nc"] = nc
    return nc


def _prepare_inputs(inputs):
    """Host-side prep: validate/normalize routing, shard over batch,
    pre-pack x feature-major fp16, replicate weights. Returns in_maps
    for the 8 cores."""
    x = np.ascontiguousarray(np.asarray(inputs["x"], dtype=np.float32))
    readout_x = np.asarray(inputs["readout_x"], dtype=np.int32)
    W1 = np.asarray(inputs["W1"], dtype=np.float32)
    W2 = np.asarray(inputs["W2"], dtype=np.float32)
    Wh = np.asarray(inputs["Wh"], dtype=np.float32)
    Ws = np.asarray(inputs["Ws"], dtype=np.float32)
    Wc = np.asarray(inputs["Wc"], dtype=np.float32)
    b1 = np.asarray(inputs["b1"], dtype=np.float32)
    b2 = np.asarray(inputs["b2"], dtype=np.float32)
    bh = np.asarray(inputs["bh"], dtype=np.float32)
    bs = np.asarray(inputs["bs"], dtype=np.float32)
    bc = np.asarray(inputs["bc"], dtype=np.float32)

    # The kernel hardcodes the cyclic PAD/brick/comb routing. If the actual
    # readout pattern differs, permute x on the host so the device sees the
    # canonical layout (mirrors jnp.nonzero(..., size=ntok) semantics).
    ntok = TS_ * B
    rf = readout_x.reshape(-1)
    canonical = np.array_equal(
        readout_x, np.broadcast_to((np.arange(S, dtype=np.int32) % 3)[:, None], (S, B))
    )
    if not canonical:
        xf = x.reshape(S * B, C)
        xc = np.zeros_like(x).reshape(S * B, C)
        for name_idx in (1, 2):
            idx = np.nonzero(rf == name_idx)[0]
            if idx.shape[0] < ntok:
                idx = np.pad(idx, (0, ntok - idx.shape[0]))
            else:
                idx = idx[:ntok]
            tgt = (3 * (np.arange(ntok) // B) + name_idx) * B + (np.arange(ntok) % B)
            xc[tgt] = xf[idx]
        x = xc.reshape(S, B, C)

    # feature-major fp16 pack: [name, core, C, t, b] with brick first, then
    # block-major cols per core: [p, ni, block, k, tok] so each (name,
    # block) is one contiguous 4KB-line DMA
    xh = x.astype(np.float16).reshape(TS_, 3, NCORES, BL, C)
    xp = xh[:, 1:3].transpose(1, 2, 4, 0, 3)  # [name, core, C, t, b]

    Wsc = np.ascontiguousarray(np.concatenate([Ws, Wc], axis=1).astype(np.float16))
    W1h = np.ascontiguousarray(W1.astype(np.float16))
    W2h = np.ascontiguousarray(W2.astype(np.float16))
    Whh = np.ascontiguousarray(Wh.astype(np.float16))
    b12 = np.ascontiguousarray(
        np.concatenate([b1.reshape(KC, 128).T, b2.reshape(KC, 128).T], axis=1)
    )
    biasA = np.concatenate([bs, bc, bh])
    biasA_b = np.ascontiguousarray(np.broadcast_to(biasA, (128, A)))

    in_maps = []
    for c in range(NCORES):
        # brick ni=0 from readout name 1, comb ni=1 from name 2
        arr = xp[:, c].reshape(2, KC, 128, TS_, BL)  # [ni, k, p, t, b]
        segs = []
        for ni in range(2):
            for bi, nt in enumerate(SCHED):
                t0 = OFFS[bi] * TPB
                seg = arr[ni, :, :, t0 : t0 + nt * TPB, :]  # [k, p, tt, b]
                segs.append(
                    seg.transpose(1, 0, 2, 3).reshape(128, KC * nt * TT)
                )
        xt_core = np.ascontiguousarray(np.concatenate(segs, axis=1))
        in_maps.append(
            {
                "xt": xt_core,
                "w1": W1h,
                "w2": W2h,
                "wh": Whh,
                "wsc": Wsc,
                "b12": b12,
                "biasA": biasA_b,
            }
        )
    return in_maps


def _run(inputs, trace=False, trace_kwargs=None):
    nc = _build()
    in_maps = _prepare_inputs(inputs)
    res = run_bass_kernel_spmd(
        nc,
        in_maps,
        list(range(NCORES)),
        trace=trace,
        **(trace_kwargs or {}),
    )
    out = np.empty((TS_, B, A), dtype=np.float32)
    for c in range(NCORES):
        out[:, c * BL : (c + 1) * BL, :] = (
            res.results[c]["out"].reshape(TS_, BL, A).astype(np.float32)
        )
    return out, res


def kernel(**inputs) -> np.ndarray:
    out, _ = _run(inputs, trace=False)
    return out


if __name__ == "__main__":
    nc = _build()
    print("built OK")



# revision 58
# speedup vs baseline: 1.0292x; 1.0087x over previous
"""Trainium2 Bass kernel for nn_AutoDecoder (moe_routing).

Reference computation (per full input):
  x: [S=3072, B=32, C=512]; rows s%3==1 are "brick" tokens, s%3==2 are
  "combined" tokens (s%3==0 PAD rows are dead). For each (timestep, batch)
  pair:
    brick:  logits[0:80]    = x_brick @ [Ws|Wc]            (+ biases)
    comb:   h = relu(relu(x_comb @ W1 + b1) @ W2 + b2)
            logits[80:1000] = h @ Wh + bh
  out: [TS=1024, B=32, A=1000]

Strategy: data-parallel over batch (4 batch entries per core, 8 cores),
weights replicated. The host pre-packs x into feature-major fp16 tiles
(one [C, ntok] panel per readout name per core), so the device does no
transposes at all: per 512-token block it streams the 2-layer MLP
(fp16 weights, fp32 PSUM accumulation, relu+bias on ACT) and then the
head matmuls with the activations as stationary operands, producing
token-major logits in PSUM. The DVE adds the output bias while
downcasting to fp16, and fully contiguous DMAs write the fp16 logits
back (host upcasts to fp32).

Scheduling, driven by trace analysis: the kernel is PE-bound mid-run
(matmul issue at the measured peak rate: 35/183/206/216ns spacings),
so all slack lives at the edges. Startup is DMA-supply-bound (3 DMA
rings at ~115-130 GB/s each, ~2.4us spin-up, ~4 outstanding DMAs per
HW ring — a 5th issue stalls the issuing engine's FIFO, so the scalar
engine carries exactly the block-0 comb x issues and nothing else
before its ACTIVATEs). The first blocks' heads — the only consumers
of wh — are deferred 2-3 blocks and the PE fills in with MLP-only
work; a dummy-matmul warmup sized to bridge the ring spin-up keeps
the PE continuously busy so the HAM clock reaches 2.4GHz by ~11.5us
with no mid-run re-throttle; the first two blocks run k-outer so
matmuls stream with chunk arrivals, and the trailing 2-wide blocks
run k-outer to avoid ACT-gated matmul pacing. x is packed block-major
on the host so blocks>=2 load each name as ONE contiguous 4KB-line
DMA (the SW-DGE queue is descriptor-bound; strided 1KB lines defeat
its coalescer). Output tiles are written in two column halves on the
two hardware DMA queues; the very last tile computes its upper half
first into a free ps_h psum bank and quarter-splits its output DMAs
across both rings to shorten the drain tail.
"""
import sys

if "/opt/trn_rl_repo" not in sys.path:
    sys.path.append("/opt/trn_rl_repo")

import numpy as np

import concourse.bass as bass
from concourse import bacc
import concourse.mybir as mybir
import concourse.tile as tile
from concourse.bass import ts
from concourse.bass_utils import run_bass_kernel_spmd

F32 = mybir.dt.float32
F16 = mybir.dt.float16
RELU = mybir.ActivationFunctionType.Relu

# problem dims (hardcoded; kernel.py must be self-contained)
S, B, C = 3072, 32, 512
TS_ = S // 3                    # 1024 timesteps
NUM_SHAPES, NUM_COLORS, N_COMBINED = 64, 16, 920
NBRICK = NUM_SHAPES + NUM_COLORS  # 80
A = NBRICK + N_COMBINED           # 1000
NCORES = 8
BL = B // NCORES                  # 4 batch entries per core
NTOKC = TS_ * BL                  # 4096 tokens per name per core
TT = 128                          # tokens per tok-tile
TPB = TT // BL                    # 32 timesteps per tok-tile
KC = C // 128                     # 4 contraction chunks

# block schedule (tok-tiles per block): big first blocks bridge the startup
# DMA supply phase; small last blocks shorten the drain tail. Shared between
# the kernel build and the host-side x pack (the DRAM x layout is
# block-major so each block's per-name load is one contiguous DMA).
SCHED = [4] * 7 + [3, 1]
OFFS = [sum(SCHED[:i]) for i in range(len(SCHED))]
assert sum(SCHED) == TS_ // TPB

_BUILD_CACHE = {}


def _build():
    if "nc" in _BUILD_CACHE:
        return _BUILD_CACHE["nc"]
    nc = bacc.Bacc("TRN2", target_bir_lowering=False, debug=False)

    # feature-major fp16 x, block-major pack: 128 partition rows (feature
    # p within chunk); cols = [name ni][block bi][chunk k][token] so one
    # (name, block) load is a single contiguous DMA with 4KB lines — the
    # SW-DGE queue is descriptor-bound and the old 1KB strided lines
    # defeated its packet coalescer.
    xt_d = nc.declare_dram_parameter("xt", [128, 2 * KC * NTOKC], F16, isOutput=False)
    w1_d = nc.declare_dram_parameter("w1", [C, C], F16, isOutput=False)
    w2_d = nc.declare_dram_parameter("w2", [C, C], F16, isOutput=False)
    wh_d = nc.declare_dram_parameter("wh", [C, N_COMBINED], F16, isOutput=False)
    wsc_d = nc.declare_dram_parameter("wsc", [C, NBRICK], F16, isOutput=False)
    b12_d = nc.declare_dram_parameter("b12", [128, 2 * KC], F32, isOutput=False)
    bA_d = nc.declare_dram_parameter("biasA", [128, A], F32, isOutput=False)
    out_d = nc.declare_dram_parameter("out", [TS_ * BL, A], F16, isOutput=True)

    with tile.TileContext(nc) as tc:
        with (
            tc.tile_pool(name="const", bufs=1) as const,
            tc.tile_pool(name="xt", bufs=4) as xt_p,
            tc.tile_pool(name="h", bufs=4) as h_p,
            tc.tile_pool(name="osb", bufs=4) as o_p,
            tc.tile_pool(name="psh", bufs=4, space=bass.MemorySpace.PSUM) as ps_h,
            tc.tile_pool(name="psc", bufs=2, space=bass.MemorySpace.PSUM) as ps_c,
        ):
            # HAM warmup: dummy matmuls at t=0 (on a memset scratch, no DMA
            # dependency) sized to keep the PE continuously busy until the
            # first real weight/x chunks land (~2.3us after the queues spin
            # up), so the HAM clock ramp completes as early as possible and
            # the PE never goes idle before the first real matmul.
            warm_src = const.tile([128, 512], F16, tag="warm")
            nc.vector.memset(warm_src[:], 0.0)
            warm = ps_h.tile([128, 512], F32, tag="hps")
            for _ in range(4):
                nc.tensor.matmul(warm[:, 0:512], warm_src[:, 0:128], warm_src[:])
            for _ in range(10):
                nc.tensor.matmul(warm[:, 0:128], warm_src[:, 0:128], warm_src[:, 0:128])
            # pre-fire the one-time ACT activation-table load so the first
            # real relu doesn't pay ~1.3us for it
            warm_act = const.tile([128, 1], F32, tag="warmact")
            nc.scalar.activation(warm_act[0:1, 0:1], warm_src[0:1, 0:1], RELU)

            # Startup DMA placement (per-HW-queue ~115-130 GB/s for 1KB-line
            # tiles; ~2.4us ring spin-up from first issue to first packet;
            # SW-DGE gpsimd queue coalesces into 4KB packets; HW queues hold
            # only ~4 outstanding DMAs, so a 5th dma_start stalls at the
            # engine's FIFO head until a transfer completes). The scalar
            # engine issues ONLY the 4 block-0 comb x loads and then nothing
            # but ACTIVATEs + out-hi DMAs — any further DMA issue on scalar
            # queues ahead of the critical RELUs in its strict FIFO and
            # stalls layer 2 (a 2.4us PE gap in the baseline trace).
            #   sync:   w1 k0..k3, w2 k0..k3, wh k0..k3, then out-lo
            #   scalar: block-0 comb x (exactly 4 issues — a 5th DMA issue
            #           would stall at the FIFO head on ring backpressure
            #           and block every ACT behind it), then out-hi
            #   gpsimd: b1/b2 (tiny), wsc, block-1 x, block-0 brick x,
            #           biasA, then per-block x prefetch only
            w1_sb = []
            w2_sb = []
            for name, dram, out_list in (("w1", w1_d, w1_sb), ("w2", w2_d, w2_sb)):
                for k in range(KC):
                    t = const.tile([128, C], F16, tag=f"{name}_{k}")
                    out_list.append(t)
            for k in range(KC):
                nc.sync.dma_start(w1_sb[k][:], w1_d[ts(k, 128), :])
            b12_sb = const.tile([128, 2 * KC], F32, tag="b12")
            nc.gpsimd.dma_start(b12_sb[:], b12_d[:, :])
            # w2 k0..k2 ride the SW queue ahead of block-1's x (land
            # ~11-13us) while k3 follows w1 on sync — layer 2 of block 0 is
            # then supplied early and depends as little as possible on the
            # sync ring, whose startup throughput jitters run to run. wsc
            # follows (not needed until the first flush ~27us).
            for k in range(3):
                nc.gpsimd.dma_start(w2_sb[k][:], w2_d[ts(k, 128), :])
            wh_sb = []
            wsc_sb = []
            for k in range(KC):
                t = const.tile([128, NBRICK], F16, tag=f"wsc_{k}")
                nc.gpsimd.dma_start(t[:], wsc_d[ts(k, 128), :])
                wsc_sb.append(t)

            def w1s(k, m):
                return w1_sb[k][:, ts(m, 128)]

            def w2s(k, m):
                return w2_sb[k][:, ts(m, 128)]

            def load_late_consts():
                # wh on sync behind w1/w2 (lands ~25.5us, needed at the
                # first finals flush ~27us); biasA DMAs are emitted on the
                # gpsimd SW queue after the early x loads
                for k in range(KC):
                    t = const.tile([128, N_COMBINED], F16, tag=f"wh_{k}")
                    nc.sync.dma_start(t[:], wh_d[ts(k, 128), :])
                    wh_sb.append(t)
                bA_sb = const.tile([128, A], F32, tag="biasA")
                return bA_sb

            # heads for block i, emitted during block i+1: token-major
            # logits straight into PSUM ([0:80] brick, [80:1000] comb),
            # DVE bias-add + fp16 downcast, contiguous DMA out.
            def finals(pb, hi_eng=None):
                # hi_eng: queue for the upper-half out DMA. During the
                # trailing 1-wide blocks the scalar FIFO is the ACT critical
                # chain, so those flushes route hi on sync; the very last
                # tile keeps scalar so the two final transfers parallelize.
                hi_eng = hi_eng or nc.scalar
                last = pb.get("last", False)
                for t in range(pb["nt"]):
                    t_last = last and t == pb["nt"] - 1
                    if t_last:
                        # final tile: its psum halves come from the ps_h
                        # pool (the MLP psums drained already) so the MMs
                        # don't wait for t0's pco to be released by the DVE;
                        # upper comb half first, so its psum stops early and
                        # its bias-add + DMA overlap the lower half's MMs.
                        pco_hi = ps_h.tile([128, 512], F32, tag="hps")
                        pco_lo = ps_h.tile([128, 512], F32, tag="hps")
                        ot = o_p.tile([128, A], F16, tag="osb")
                        r0 = (pb["t0"] + t * TPB) * BL
                        for k in range(KC):
                            nc.tensor.matmul(
                                pco_hi[:, 0 : A - 512],
                                pb["h2"][k][:, ts(t, 128)],
                                wh_sb[k][:, 512 - NBRICK : N_COMBINED],
                                start=(k == 0),
                                stop=(k == KC - 1),
                            )
                        nc.vector.tensor_add(
                            ot[:, 512:A], pco_hi[:, 0 : A - 512], bA_sb[:, 512:A]
                        )
                        # quarter-split out DMAs across both rings so the
                        # post-last-matmul transfer is ~64KB per ring, not
                        # one 128KB straggler on a single ring
                        nc.scalar.dma_start(out_d[r0 : r0 + TT, 512:768], ot[:, 512:768])
                        nc.sync.dma_start(out_d[r0 : r0 + TT, 768:A], ot[:, 768:A])
                        for k in range(KC):
                            nc.tensor.matmul(
                                pco_lo[:, 0:NBRICK],
                                pb["xtb"][k][:, ts(t, 128)],
                                wsc_sb[k][:],
                                start=(k == 0),
                                stop=(k == KC - 1),
                            )
                        for k in range(KC):
                            nc.tensor.matmul(
                                pco_lo[:, NBRICK:512],
                                pb["h2"][k][:, ts(t, 128)],
                                wh_sb[k][:, 0 : 512 - NBRICK],
                                start=(k == 0),
                                stop=(k == KC - 1),
                            )
                        # lo half in quarter adds so each ring's final DMA
                        # issues as soon as its quarter is biased
                        nc.vector.tensor_add(
                            ot[:, 0:256], pco_lo[:, 0:256], bA_sb[:, 0:256]
                        )
                        nc.scalar.dma_start(out_d[r0 : r0 + TT, 0:256], ot[:, 0:256])
                        nc.vector.tensor_add(
                            ot[:, 256:512], pco_lo[:, 256:512], bA_sb[:, 256:512]
                        )
                        nc.sync.dma_start(out_d[r0 : r0 + TT, 256:512], ot[:, 256:512])
                        continue
                    pco = ps_c.tile([128, 1024], F32, tag="combo")
                    for k in range(KC):
                        lhs = pb["h2"][k][:, ts(t, 128)]
                        nc.tensor.matmul(
                            pco[:, NBRICK:512],
                            lhs,
                            wh_sb[k][:, 0 : 512 - NBRICK],
                            start=(k == 0),
                            stop=(k == KC - 1),
                        )
                        nc.tensor.matmul(
                            pco[:, 512:A],
                            lhs,
                            wh_sb[k][:, 512 - NBRICK : N_COMBINED],
                            start=(k == 0),
                            stop=(k == KC - 1),
                        )
                    for k in range(KC):
                        nc.tensor.matmul(
                            pco[:, 0:NBRICK],
                            pb["xtb"][k][:, ts(t, 128)],
                            wsc_sb[k][:],
                            start=(k == 0),
                            stop=(k == KC - 1),
                        )
                    # bias-add + fp16 downcast in two column halves so the
                    # first half's DMA fires while the DVE does the second
                    # half; halves go to different queues
                    ot = o_p.tile([128, A], F16, tag="osb")
                    r0 = (pb["t0"] + t * TPB) * BL
                    nc.vector.tensor_add(ot[:, 0:512], pco[:, 0:512], bA_sb[:, 0:512])
                    nc.sync.dma_start(out_d[r0 : r0 + TT, 0:512], ot[:, 0:512])
                    nc.vector.tensor_add(ot[:, 512:A], pco[:, 512:A], bA_sb[:, 512:A])
                    hi_eng.dma_start(out_d[r0 : r0 + TT, 512:A], ot[:, 512:A])

            # ---- main loop over blocks of tok-tiles (128 tokens each) ----
            sched = SCHED
            offs = OFFS
            # heads deferral: the first ~25us are DMA-limited (weights still
            # streaming in at ~60-80 GB/s per queue), so the first blocks'
            # heads — which need wh — are pushed back 1-2 extra blocks; the
            # PE fills that time with MLP work that only needs w1/w2/x. The
            # backlog is worked off two-per-block once wh has landed.
            nblk = len(sched)
            flush_n = {2: 1, 3: 2}
            flush_n.update({i: 1 for i in range(4, nblk)})

            # feature-major fp16 x loads, issued one block AHEAD so the
            # queue has a full block period to deliver. Block 0's comb x
            # goes on the scalar HW queue (parallel with w1 on sync) and
            # its brick x on gpsimd; later blocks use the gpsimd
            # software-DGE queue entirely. Blocks 0/1 split per k-chunk
            # (compute streams with chunk arrivals; block 0's first chunk
            # is halved again for the earliest possible first matmul);
            # later blocks load each name as ONE contiguous 4KB-line DMA.
            def issue_name_loads(bi, ni, eng):
                nt = sched[bi]
                W_ = nt * TT
                c0 = ni * (KC * NTOKC) + offs[bi] * TT * KC
                tl = xt_p.tile([128, KC * W_], F16, tag=f"xt{ni}")
                if bi == 0 and ni == 1:
                    eng.dma_start(tl[:, 0:W_], xt_d[:, c0 : c0 + W_])
                    eng.dma_start(tl[:, W_ : 2 * W_], xt_d[:, c0 + W_ : c0 + 2 * W_])
                    eng.dma_start(
                        tl[:, 2 * W_ : 4 * W_], xt_d[:, c0 + 2 * W_ : c0 + 4 * W_]
                    )
                elif bi == 1:
                    for k in range(KC):
                        eng.dma_start(
                            tl[:, k * W_ : (k + 1) * W_],
                            xt_d[:, c0 + k * W_ : c0 + (k + 1) * W_],
                        )
                else:
                    eng.dma_start(tl[:], xt_d[:, c0 : c0 + KC * W_])
                return [tl[:, k * W_ : (k + 1) * W_] for k in range(KC)]

            def issue_loads(bi):
                return (
                    issue_name_loads(bi, 1, nc.gpsimd),
                    issue_name_loads(bi, 0, nc.gpsimd),
                )

            xtc0 = issue_name_loads(0, 1, nc.scalar)
            for k in range(3, KC):
                nc.sync.dma_start(w2_sb[k][:], w2_d[ts(k, 128), :])
            bA_sb = load_late_consts()
            issued = {1: issue_loads(1)}
            # block 0's brick x goes behind block 1's x on the SW queue —
            # it is not read until the deferred heads flush
            issued[0] = (xtc0, issue_name_loads(0, 0, nc.gpsimd))
            # biasA rides the deep SW-DGE ring behind the early x loads
            # (lands ~22us, first bias-add needs it ~27us); the HW rings
            # would either stall ACTs (scalar) or push wh too late (sync)
            nc.gpsimd.dma_start(bA_sb[:, 0:512], bA_d[:, 0:512])
            nc.gpsimd.dma_start(bA_sb[:, 512:A], bA_d[:, 512:A])
            pendings = []
            for bi, nt in enumerate(sched):
                ti0 = offs[bi]
                t0 = ti0 * TPB
                W_ = nt * TT  # tokens per name in this block
                xtc, xtb = issued.pop(bi)
                if bi + 1 < len(sched) and bi + 1 not in issued:
                    issued[bi + 1] = issue_loads(bi + 1)

                # deferred heads of earlier blocks (see note above); for
                # the trailing blocks they are emitted BETWEEN the two MLP
                # layers (below) so the PE covers the h1 ACT latency at the
                # layer boundary instead of idling ~290ns per boundary
                flushes = [pendings.pop(0) for _ in range(flush_n.get(bi, 0))]
                late = bi >= nblk - 2
                if not late:
                    for pb in flushes:
                        finals(pb)

                # comb MLP: hT[m] = relu(W[:,m-chunk].T @ inT + b). The first
                # two blocks run k-outer so matmuls stream with the arriving
                # weight/x DMA chunks; the trailing 1-wide blocks run k-outer
                # too (m-outer at W=128 is ACT-gated: ~190ns/MM instead of
                # 56); mid-run 4-wide blocks run m-outer so the ACT engine
                # drains each m-psum while the next accumulates.
                def layer(ws, inp, btag, hname):
                    hs_list = []
                    if bi < 2 or bi >= nblk - 2:
                        phs = []
                        for _m in range(KC):
                            ph = ps_h.tile([128, W_], F32, tag="hps")
                            phs.append(ph)
                        # NOTE: do not split a k-stage into column sub-ranges
                        # of one psum bank — a bank supports only ONE open
                        # accumulation group at a time, and a second start=
                        # True group in the same bank corrupts the first.
                        for k in range(KC):
                            for m in range(KC):
                                nc.tensor.matmul(
                                    phs[m][:],
                                    ws(k, m),
                                    inp[k][:],
                                    start=(k == 0),
                                    stop=(k == KC - 1),
                                )
                        for m in range(KC):
                            hs = h_p.tile([128, W_], F16, tag=f"{hname}_{m}")
                            nc.scalar.activation(
                                hs[:],
                                phs[m][:],
                                RELU,
                                bias=b12_sb[:, btag + m : btag + m + 1],
                                scale=1.0,
                            )
                            hs_list.append(hs)
                    else:
                        for m in range(KC):
                            ph = ps_h.tile([128, W_], F32, tag="hps")
                            for k in range(KC):
                                nc.tensor.matmul(
                                    ph[:],
                                    ws(k, m),
                                    inp[k][:],
                                    start=(k == 0),
                                    stop=(k == KC - 1),
                                )
                            hs = h_p.tile([128, W_], F16, tag=f"{hname}_{m}")
                            nc.scalar.activation(
                                hs[:],
                                ph[:],
                                RELU,
                                bias=b12_sb[:, btag + m : btag + m + 1],
                                scale=1.0,
                            )
                            hs_list.append(hs)
                    return hs_list

                h1 = layer(w1s, xtc, 0, "h1")
                if late:
                    for pb in flushes:
                        finals(pb, hi_eng=nc.sync)
                h2 = layer(w2s, h1, KC, "h2")

                pendings.append({"h2": h2, "xtb": xtb, "t0": t0, "nt": nt})
            assert len(pendings) == 1
            pendings[0]["last"] = True
            # keep-warm: the out rings have been idle for ~5us by now; a
            # tiny read on each re-spins them so the final out transfers
            # don't pay the ring restart latency
            kw = const.tile([1, C], F16, tag="kw")
            nc.sync.dma_start(kw[0:1, 0:256], w1_d[0:1, 0:256])
            nc.scalar.dma_start(kw[0:1, 256:512], w1_d[0:1, 256:512])
            finals(pendings[0])

    nc.compile()
    _BUILD_CACHE["nc"] = nc
    return nc


def _prepare_inputs(inputs):
    """Host-side prep: validate/normalize routing, shard over batch,
    pre-pack x feature-major fp16, replicate weights. Returns in_maps
    for the 8 cores."""
    x = np.ascontiguousarray(np.asarray(inputs["x"], dtype=np.float32))
    readout_x = np.asarray(inputs["readout_x"], dtype=np.int32)
    W1 = np.asarray(inputs["W1"], dtype=np.float32)
    W2 = np.asarray(inputs["W2"], dtype=np.float32)
    Wh = np.asarray(inputs["Wh"], dtype=np.float32)
    Ws = np.asarray(inputs["Ws"], dtype=np.float32)
    Wc = np.asarray(inputs["Wc"], dtype=np.float32)
    b1 = np.asarray(inputs["b1"], dtype=np.float32)
    b2 = np.asarray(inputs["b2"], dtype=np.float32)
    bh = np.asarray(inputs["bh"], dtype=np.float32)
    bs = np.asarray(inputs["bs"], dtype=np.float32)
    bc = np.asarray(inputs["bc"], dtype=np.float32)

    # The kernel hardcodes the cyclic PAD/brick/comb routing. If the actual
    # readout pattern differs, permute x on the host so the device sees the
    # canonical layout (mirrors jnp.nonzero(..., size=ntok) semantics).
    ntok = TS_ * B
    rf = readout_x.reshape(-1)
    canonical = np.array_equal(
        readout_x, np.broadcast_to((np.arange(S, dtype=np.int32) % 3)[:, None], (S, B))
    )
    if not canonical:
        xf = x.reshape(S * B, C)
        xc = np.zeros_like(x).reshape(S * B, C)
        for name_idx in (1, 2):
            idx = np.nonzero(rf == name_idx)[0]
            if idx.shape[0] < ntok:
                idx = np.pad(idx, (0, ntok - idx.shape[0]))
            else:
                idx = idx[:ntok]
            tgt = (3 * (np.arange(ntok) // B) + name_idx) * B + (np.arange(ntok) % B)
            xc[tgt] = xf[idx]
        x = xc.reshape(S, B, C)

    # feature-major fp16 pack: [name, core, C, t, b] with brick first, then
    # block-major cols per core: [p, ni, block, k, tok] so each (name,
    # block) is one contiguous 4KB-line DMA
    xh = x.astype(np.float16).reshape(TS_, 3, NCORES, BL, C)
    xp = xh[:, 1:3].transpose(1, 2, 4, 0, 3)  # [name, core, C, t, b]

    Wsc = np.ascontiguousarray(np.concatenate([Ws, Wc], axis=1).astype(np.float16))
    W1h = np.ascontiguousarray(W1.astype(np.float16))
    W2h = np.ascontiguousarray(W2.astype(np.float16))
    Whh = np.ascontiguousarray(Wh.astype(np.float16))
    b12 = np.ascontiguousarray(
        np.concatenate([b1.reshape(KC, 128).T, b2.reshape(KC, 128).T], axis=1)
    )
    biasA = np.concatenate([bs, bc, bh])
    biasA_b = np.ascontiguousarray(np.broadcast_to(biasA, (128, A)))

    in_maps = []
    for c in range(NCORES):
        # brick ni=0 from readout name 1, comb ni=1 from name 2
        arr = xp[:, c].reshape(2, KC, 128, TS_, BL)  # [ni, k, p, t, b]
        segs = []
        for ni in range(2):
            for bi, nt in enumerate(SCHED):
                t0 = OFFS[bi] * TPB
                seg = arr[ni, :, :, t0 : t0 + nt * TPB, :]  # [k, p, tt, b]
                segs.append(
                    seg.transpose(1, 0, 2, 3).reshape(128, KC * nt * TT)
                )
        xt_core = np.ascontiguousarray(np.concatenate(segs, axis=1))
        in_maps.append(
            {
                "xt": xt_core,
                "w1": W1h,
                "w2": W2h,
                "wh": Whh,
                "wsc": Wsc,
                "b12": b12,
                "biasA": biasA_b,
            }
        )
    return in_maps


def _run(inputs, trace=False, trace_kwargs=None):
    nc = _build()
    in_maps = _prepare_inputs(inputs)
    res = run_bass_kernel_spmd(
        nc,
        in_maps,
        list(range(NCORES)),
        trace=trace,
        **(trace_kwargs or {}),
    )
    out = np.empty((TS_, B, A), dtype=np.float32)
    for c in range(NCORES):
        out[:, c * BL : (c + 1) * BL, :] = (
            res.results[c]["out"].reshape(TS_, BL, A).astype(np.float32)
        )
    return out, res


def kernel(**inputs) -> np.ndarray:
    out, _ = _run(inputs, trace=False)
    return out


if __name__ == "__main__":
    nc = _build()
    print("built OK")

